# revision 37
# baseline (speedup 1.0000x reference)
"""Deformable-DETR transformer encoder (3 layers) on 8 Trainium2 NeuronCores.

Sharding: core c -> (batch b = c//4, query-quarter r = c%4). Each core
processes 2550 queries (padded to 2560) of one batch, all 8 heads.
Per layer the value projection is computed on the owned quarter and
all-gathered (groups of 4 cores) so every core can sample anywhere.

Sampling: for each (query, head, level, point) the 4 bilinear corners are
fetched with ONE dma_gather descriptor from a "quad" value table
valP4[pos] = [v(pos), v(pos+1), v(pos+W), v(pos+W+1)] (bf16, 256B rows),
then combined with hat-function weights (folding bilinear weights and the
attention softmax) on the vector engine.

Host<->device traffic is minimized (the axon tunnel moves ~70 MB/s):
 - src ships as per-row int8 + f32 scales (dequantized on device); the
   output is quantized per row on device (uint8 + f32 scales) and
   dequantized on the host, shard-by-shard while later shards stream.
 - pos ships once as f16 (device-cached); pos^T is computed on device, and
   q^T = x^T + pos^T per layer, so no host-side pos@W projection ships.
 - all GEMM biases are folded into the PSUM accumulation as rank-1
   (ones x bias_row) matmuls; LN params are broadcast on device.
 - weights are device-resident across kernel() calls (fingerprint-checked),
   and the jitted executable is cached, mirroring
   bass_utils.run_bass_kernel_spmd's axon path (run_bass_via_pjrt) minus
   the per-call retrace.
"""

import hashlib
import numpy as np
import ml_dtypes

# ---------------- problem constants (hardcoded) ----------------
LEVEL_SHAPES = ((48, 160), (24, 80), (12, 40), (6, 20))
LEN = sum(h * w for h, w in LEVEL_SHAPES)  # 10200
B, D, NH, NL, NP, DFF, NLAYERS = 2, 256, 8, 4, 4, 1024, 3
DH = D // NH  # 32
LEVEL_START = [0]
for _h, _w in LEVEL_SHAPES[:-1]:
    LEVEL_START.append(LEVEL_START[-1] + _h * _w)

NCORES = 8
Q = LEN // 4          # 2550 queries per core
QP = 2560             # padded
T = QP // 128         # 20 query tiles
VF_ROWS = LEN + LEVEL_SHAPES[-1][1] + 1   # val_full rows incl. pad (10221)
NJ = 128              # samples per query: j = (h 8, lvl 4, p 4)
BF16 = ml_dtypes.bfloat16

_NC_CACHE = {}
_SESSION = {}

# inputs that change every call; everything else (weights, pos embedding,
# valid_ratios-derived tables) is device-cached behind a content fingerprint
_DYN_NAMES = ("xq",)


class _K:
    """Holds builder state shared across helper functions."""
    pass


def _ln(K, pool, u, out_ap, g, b):
    nc, Alu, Act, Ax, F32 = K.nc, K.Alu, K.Act, K.Ax, K.F32
    m = pool.tile([128, 1], F32, tag="ln_m")
    nc.vector.tensor_reduce(m[:], u[:], Ax.X, Alu.add)
    nc.vector.tensor_scalar(m[:], m[:], 1.0 / 256.0, None, Alu.mult)
    c = pool.tile([128, 256], F32, tag="ln_c")
    nc.vector.tensor_scalar(c[:], u[:], m[:], None, Alu.subtract)
    scr = pool.tile([128, 256], F32, tag="ln_scr")
    v = pool.tile([128, 1], F32, tag="ln_v")
    nc.scalar.activation(scr[:], c[:], Act.Square, accum_out=v[:])
    nc.vector.tensor_scalar(v[:], v[:], 1.0 / 256.0, 1e-5, Alu.mult, Alu.add)
    nc.scalar.activation(v[:], v[:], Act.Sqrt)
    nc.vector.reciprocal(v[:], v[:])
    nc.vector.scalar_tensor_tensor(out_ap, c[:], v[:], g, Alu.mult, Alu.mult)
    nc.vector.tensor_tensor(out_ap, out_ap, b, Alu.add)


def _hats(K, cc, bf, h0, h1, dd):
    # h0 = relu(1-|c-b|), h1 = relu(1-|c-b-1|)
    nc, Alu = K.nc, K.Alu
    nc.vector.tensor_tensor(dd[:], cc[:], bf[:], Alu.subtract)
    nc.vector.tensor_scalar(h0[:], dd[:], -1.0, None, Alu.mult)
    nc.vector.tensor_tensor(h0[:], h0[:], dd[:], Alu.max)
    nc.vector.tensor_scalar(h0[:], h0[:], -1.0, 1.0, Alu.mult, Alu.add)
    nc.vector.tensor_scalar(h0[:], h0[:], 0.0, None, Alu.max)
    nc.vector.tensor_scalar(dd[:], dd[:], -1.0, None, Alu.add)
    nc.vector.tensor_scalar(h1[:], dd[:], -1.0, None, Alu.mult)
    nc.vector.tensor_tensor(h1[:], h1[:], dd[:], Alu.max)
    nc.vector.tensor_scalar(h1[:], h1[:], -1.0, 1.0, Alu.mult, Alu.add)
    nc.vector.tensor_scalar(h1[:], h1[:], 0.0, None, Alu.max)


def _weight_calc(K, t, offa, w2, idx_all, tl):
    """Per-sample sampling weights + gather indices for query tile t."""
    nc, Alu, Act, Ax = K.nc, K.Alu, K.Act, K.Ax
    F32, I32 = K.F32, K.I32
    wcp, rxy = K.wcp, K.rxy

    def off_ap(xy):
        return offa[:, 0:256].rearrange(
            "q (h lvl p two) -> q h lvl p two", h=8, lvl=4, p=4, two=2)[:, :, :, :, xy]

    def rxy_ap(xy):
        a = rxy[:, t, :].rearrange("q (lvl two) -> q lvl two", lvl=4)[:, :, xy]
        return a.unsqueeze(1).broadcast_to([128, 8, 4]).unsqueeze(3).broadcast_to([128, 8, 4, 4])

    jv = "q (h lvl p) -> q h lvl p"
    cx = wcp.tile([128, NJ], F32, tag="cx")
    cy = wcp.tile([128, NJ], F32, tag="cy")
    nc.vector.tensor_tensor(cx.rearrange(jv, h=8, lvl=4), off_ap(0), rxy_ap(0), Alu.add)
    nc.vector.tensor_tensor(cy.rearrange(jv, h=8, lvl=4), off_ap(1), rxy_ap(1), Alu.add)

    bxi = wcp.tile([128, NJ], I32, tag="bxi")
    byi = wcp.tile([128, NJ], I32, tag="byi")
    nc.vector.tensor_copy(bxi[:], cx[:])   # trunc cast
    nc.vector.tensor_copy(byi[:], cy[:])
    nc.vector.tensor_scalar(bxi[:], bxi[:], 0, None, Alu.max)
    nc.vector.tensor_scalar(byi[:], byi[:], 0, None, Alu.max)
    nc.vector.tensor_tensor(bxi[:], bxi[:], K.jWM2[:], Alu.min)
    nc.vector.tensor_tensor(byi[:], byi[:], K.jHM2[:], Alu.min)
    bxf = wcp.tile([128, NJ], F32, tag="bxf")
    byf = wcp.tile([128, NJ], F32, tag="byf")
    nc.vector.tensor_copy(bxf[:], bxi[:])
    nc.vector.tensor_copy(byf[:], byi[:])

    hx0 = wcp.tile([128, NJ], F32, tag="hx0")
    hx1 = wcp.tile([128, NJ], F32, tag="hx1")
    hy0 = wcp.tile([128, NJ], F32, tag="hy0")
    hy1 = wcp.tile([128, NJ], F32, tag="hy1")
    dd = wcp.tile([128, NJ], F32, tag="dd")
    _hats(K, cx, bxf, hx0, hx1, dd)
    _hats(K, cy, byf, hy0, hy1, dd)

    # attention softmax over (lvl,p) per head
    ex = wcp.tile([128, 128], F32, tag="ex")
    nc.scalar.activation(ex[:], offa[:, 256:384], Act.Exp)
    es = wcp.tile([128, 8], F32, tag="es")
    nc.vector.tensor_reduce(es[:], ex.rearrange("q (h f) -> q h f", h=8), Ax.X, Alu.add)
    er = wcp.tile([128, 8], F32, tag="er")
    nc.vector.reciprocal(er[:], es[:])
    a2 = wcp.tile([128, 128], F32, tag="a2")
    nc.vector.tensor_tensor(
        a2.rearrange("q (h f) -> q h f", h=8),
        ex.rearrange("q (h f) -> q h f", h=8),
        er.unsqueeze(2).broadcast_to([128, 8, 16]), Alu.mult)

    wy0 = wcp.tile([128, NJ], F32, tag="wy0")
    wy1 = wcp.tile([128, NJ], F32, tag="wy1")
    nc.vector.tensor_tensor(wy0[:], hy0[:], a2[:], Alu.mult)
    nc.vector.tensor_tensor(wy1[:], hy1[:], a2[:], Alu.mult)

    # w2[q, tl, j*8+s*2+dup] = wy_sy * hx_sx   (s = sy*2+sx)
    for sy, wyv in ((0, wy0), (1, wy1)):
        for sx, hxv in ((0, hx0), (1, hx1)):
            outap = w2[:, tl, :].rearrange("q (j s dup) -> q j s dup", j=NJ, s=4)[:, :, sy * 2 + sx, :]
            nc.vector.tensor_tensor(
                outap, wyv.unsqueeze(2).broadcast_to([128, NJ, 2]),
                hxv.unsqueeze(2).broadcast_to([128, NJ, 2]), Alu.mult)

    # idx = ((LS + by*W + bx) << 1) + hp   (jLS2H = 2*LS+hp)
    nc.vector.tensor_tensor(byi[:], byi[:], K.jW[:], Alu.mult)
    nc.vector.tensor_tensor(byi[:], byi[:], bxi[:], Alu.add)
    nc.vector.tensor_scalar(byi[:], byi[:], 1, None, Alu.logical_shift_left)
    nc.vector.tensor_tensor(byi[:], byi[:], K.jLS2H[:], Alu.add)
    nc.vector.tensor_copy(idx_all[:, tl], byi[:])


def _transpose_set(K, src3, t, dst, identity, psum_tag):
    """PE-transpose src3[:, t, k*128:(k+1)*128] into dst[:, k, t*128:...] for k=0,1."""
    nc = K.nc
    for k in range(2):
        pt = K.psT.tile([128, 128], identity.dtype, tag=psum_tag)
        nc.tensor.transpose(pt[:], src3[:, t, k * 128:(k + 1) * 128], identity[:])
        nc.scalar.copy(dst[:, k, t * 128:(t + 1) * 128], pt[:])


def _layer(K, layer, x, dbg_on):
    nc, Alu, Act = K.nc, K.Alu, K.Act
    F32, F32R, BF, F16, I16 = K.F32, K.F32R, K.BF, K.F16, K.I16
    dbg = K.dbg

    # ---- per-layer weights ----
    wlp, brp = K.wlp, K.brp
    woa = wlp.tile([128, 2, 384], F32R, tag="woa")
    nc.sync.dma_start(woa[:], K.woa_in[layer].rearrange("k p n -> p k n"))
    wval = wlp.tile([128, 2, D], F32R, tag="wval")
    nc.sync.dma_start(wval[:], K.wval_in[layer].rearrange("k p n -> p k n"))
    wout = wlp.tile([128, 2, D], BF, tag="wout")
    nc.sync.dma_start(wout[:], K.wout_in[layer].rearrange("k p n -> p k n"))
    wff1 = wlp.tile([128, 2, DFF], F32R, tag="wff1")
    nc.sync.dma_start(wff1[:], K.wff1_in[layer].rearrange("k p n -> p k n"))
    wff2 = wlp.tile([128, 8, D], BF, tag="wff2")
    nc.sync.dma_start(wff2[:], K.wff2_in[layer].rearrange("k p n -> p k n"))

    # bias rows (added via rank-1 ones x row matmuls inside PSUM groups)
    boa_l = brp.tile([1, 384], F32R, tag="boa")
    nc.sync.dma_start(boa_l[:], K.boa_in[layer])
    bvr = brp.tile([1, D], F32R, tag="bvr")
    nc.sync.dma_start(bvr[:], K.bvr_in[layer])
    bor = brp.tile([1, D], BF, tag="bor")
    nc.sync.dma_start(bor[:], K.bor_in[layer])
    bf2r = brp.tile([1, D], BF, tag="bf2r")
    nc.sync.dma_start(bf2r[:], K.bf2r_in[layer])
    lnr = brp.tile([1, 1024], F32R, tag="lnr")
    nc.sync.dma_start(lnr[:], K.lnrow_in[layer])
    bff1 = brp.tile([128, 8], F32, tag="bff1")
    nc.sync.dma_start(bff1[:], K.bff1_in[layer])

    # LN params broadcast to all 128 partitions: lngb = ones x (g1|b1|g2|b2)
    lngb = wlp.tile([128, 1024], F32, tag="lngb")
    for hh in range(2):
        pl = K.psA.tile([128, 512], F32, tag="gemm")
        nc.tensor.matmul(pl[:], K.ones1r[:], lnr[:, hh * 512:(hh + 1) * 512],
                         start=True, stop=True)
        nc.scalar.copy(lngb[:, hh * 512:(hh + 1) * 512], pl[:])
    g1, b1 = lngb[:, 0:256], lngb[:, 256:512]
    g2, b2 = lngb[:, 512:768], lngb[:, 768:1024]

    # ---- x^T ----
    xT = K.xtp.tile([128, 2, QP], F32R, tag="xT")
    for t in range(T):
        _transpose_set(K, x, t, xT, K.ident, "tp")

    # ---- val GEMM -> bounce -> AllGather -> valP4 ----
    vbounce = K.dram.tile([Q, D], BF, tag="vb")
    for t in range(T):
        pv = K.psA.tile([128, D], F32, tag="gemm")
        ts = slice(t * 128, (t + 1) * 128)
        nc.tensor.matmul(pv[:], xT[:, 0, ts], wval[:, 0], start=True, stop=False)
        nc.tensor.matmul(pv[:], xT[:, 1, ts], wval[:, 1], start=False, stop=False)
        nc.tensor.matmul(pv[:], K.ones1r[:], bvr[:], start=False, stop=True)
        sval = K.wkp.tile([128, D], BF, tag="sval")
        nc.scalar.copy(sval[:], pv[:])
        nrows = min(128, Q - t * 128)
        nc.sync.dma_start(vbounce[t * 128:t * 128 + nrows, :], sval[:nrows, :])
    valfull = K.dram.tile([VF_ROWS, D], BF, tag="vf")
    if "nocoll" in K.bisect:
        for rr in range(4):
            nc.sync.dma_start(valfull[rr * Q:(rr + 1) * Q, :], vbounce[:])
    else:
        nc.gpsimd.collective_compute(
            "AllGather", Alu.bypass, replica_groups=K.groups,
            ins=[vbounce[:].opt()], outs=[valfull[0:LEN, :].opt()])
    if dbg_on:
        nc.sync.dma_start(dbg["vf"][:], valfull[0:LEN, :])

    # valP4[h2][pos*2+hp] = [v(pos), v(pos+1), v(pos+W), v(pos+W+1)] of head h2*2+hp
    valP4 = [K.dramP.tile([2 * VF_ROWS, 128], BF, tag=f"vp{h2}", name=f"valP4_{h2}") for h2 in range(4)]
    for h2 in range(4):
        for lvl, (H, W) in enumerate(LEVEL_SHAPES):
            npos = H * W
            base = LEVEL_START[lvl]
            for c, dc in enumerate((0, 1, W, W + 1)):
                src = valfull[base + dc: base + dc + npos,
                              h2 * 64:(h2 + 1) * 64].rearrange("pos (hp ch) -> pos hp ch", hp=2)
                dst = valP4[h2][2 * base: 2 * (base + npos),
                                c * 32:(c + 1) * 32].rearrange("(pos hp) ch -> pos hp ch", hp=2)
                nc.sync.dma_start(dst, src)

    # ---- off/attn GEMM + weight calc + idx + table shuffle (2 halves) ----
    w2h, tabh = [], []
    for half in range(2):
        w2 = K.w2p.tile([128, 10, 1024], BF, tag="w2")
        idx_all = K.w2p.tile([128, 10, NJ], I16, tag="idx")
        for tl in range(10):
            t = half * 10 + tl
            ts = slice(t * 128, (t + 1) * 128)
            # q^T tile = x^T + pos^T (pos projection happens here on device)
            pTs = K.wkp.tile([128, 2, 128], F16, tag="pTs")
            nc.sync.dma_start(pTs[:], K.posT_d[:, :, ts])
            qTt = K.wkp.tile([128, 2, 128], F32R, tag="qTt")
            nc.vector.tensor_tensor(qTt[:], xT[:, :, ts], pTs[:], Alu.add)
            po = K.psA.tile([128, 384], F32, tag="gemm")
            nc.tensor.matmul(po[:], qTt[:, 0], woa[:, 0], start=True, stop=False)
            nc.tensor.matmul(po[:], qTt[:, 1], woa[:, 1], start=False, stop=False)
            nc.tensor.matmul(po[:], K.ones1r[:], boa_l[:], start=False, stop=True)
            offa = K.wkp.tile([128, 384], F32, tag="offa")
            nc.scalar.copy(offa[:], po[:])
            if dbg_on:
                nc.sync.dma_start(dbg["offa"][:, t], offa[:])
            _weight_calc(K, t, offa, w2, idx_all, tl)
        if dbg_on:
            nc.sync.dma_start(dbg["w2"][:, half * 10:(half + 1) * 10], w2[:])
            nc.sync.dma_start(dbg["idx"][:, half * 10:(half + 1) * 10], idx_all[:])

        for qt in range(2):
            tb = K.tbp.tile([128, 5 * 1024], I16, tag="tb", name=f"tb_{half}_{qt}")
            for qhi in range(8):
                src = idx_all[qhi * 16:(qhi + 1) * 16, qt * 5:(qt + 1) * 5, :].rearrange(
                    "q tl (h2 bb) -> q tl h2 bb", h2=4)
                dst = tb[0:16, :].rearrange("q (tl h2 bb qhi) -> q tl h2 bb qhi",
                                            tl=5, h2=4, bb=32)[:, :, :, :, qhi]
                nc.sync.dma_start(dst, src)
            for rep in range(1, 8):
                nc.sync.dma_start(tb[rep * 16:(rep + 1) * 16, :], tb[0:16, :])
            tabh.append(tb)
        w2h.append(w2)

    # ---- gather + weighting -> attn_out -> aoT ----
    aoT = K.xtp.tile([128, 2, QP], BF, tag="aoT")
    for t in range(T):
        ao = K.wkp.tile([128, D], BF, tag="ao")
        tb, w2, tl = tabh[t // 5], w2h[t // 10], t % 10
        tq = t % 5
        for h2 in range(4):
            G = K.gp.tile([128, 32, 128], BF, tag="G")
            if "nogather" in K.bisect:
                nc.gpsimd.memset(G[:], 0.25)
            else:
                for q4 in range(4):
                    co = tq * 1024 + h2 * 256 + q4 * 64
                    nc.gpsimd.dma_gather(
                        G[:, q4 * 8:(q4 + 1) * 8, :], valP4[h2][:],
                        tb[:, co: co + 64],
                        num_idxs=1024, num_idxs_reg=1024, elem_size=128,
                        queue_num=q4)
            tmp = K.tp.tile([128, 4096], BF, tag="tmp")
            g_ap = G[:].rearrange("q b e -> q (b e)").rearrange("q (g ch) -> q g ch", ch=32)
            w_ap = w2[:, tl, h2 * 256:(h2 + 1) * 256].rearrange(
                "q (g dup) -> q g dup", dup=2).unsqueeze(2).broadcast_to([128, 128, 16, 2])
            nc.vector.tensor_tensor(tmp.rearrange("q (g ch) -> q g ch", ch=32), g_ap, w_ap, Alu.mult)
            # tree reduce over (lvl, p, s) keeping (hp, ch); layout (hp 2, lvl 4, p 4, s 4, ch 32)
            cur, n = tmp, 2048
            for _ in range(6):
                nxt = K.tp.tile([128, n], BF, tag=f"r{n}")
                va = cur.rearrange("q (hp f) -> q hp f", hp=2)
                nc.vector.tensor_tensor(
                    nxt.rearrange("q (hp f) -> q hp f", hp=2),
                    va[:, :, 0:n // 2], va[:, :, n // 2:n], Alu.add)
                cur, n = nxt, n // 2
            nc.vector.tensor_copy(ao[:, h2 * 64:(h2 + 1) * 64], cur[:])
        if dbg_on:
            nc.sync.dma_start(dbg["ao"][:, t], ao[:])
        _transpose_set_src2(K, ao, t, aoT, K.identb, "tpb")

    # ---- out proj + residual + LN1 ----
    x2 = K.xsp.tile([128, T, D], F32, tag="x")
    for t in range(T):
        po = K.psA.tile([128, D], F32, tag="gemm")
        ts = slice(t * 128, (t + 1) * 128)
        nc.tensor.matmul(po[:], aoT[:, 0, ts], wout[:, 0], start=True, stop=False)
        nc.tensor.matmul(po[:], aoT[:, 1, ts], wout[:, 1], start=False, stop=False)
        nc.tensor.matmul(po[:], K.ones1b[:], bor[:], start=False, stop=True)
        u = K.wkp.tile([128, D], F32, tag="u")
        nc.vector.tensor_tensor(u[:], po[:], x[:, t], Alu.add)
        _ln(K, K.wkp, u, x2[:, t], g1, b1)
        if dbg_on:
            nc.sync.dma_start(dbg["x1"][:, t], x2[:, t])

    # ---- FFN (chunked over 512 queries) ----
    x2T = K.xtp.tile([128, 2, QP], F32R, tag="xT")
    for t in range(T):
        _transpose_set(K, x2, t, x2T, K.ident, "tp")
    xn = K.xsp.tile([128, T, D], F32, tag="x")
    for ch in range(5):
        h1c = K.h1p.tile([128, 8, 512], BF, tag="h1c")
        cs = slice(ch * 512, (ch + 1) * 512)
        for ot in range(8):
            ph = K.psA.tile([128, 512], F32, tag="gemm")
            os_ = slice(ot * 128, (ot + 1) * 128)
            nc.tensor.matmul(ph[:], wff1[:, 0, os_], x2T[:, 0, cs], start=True, stop=False)
            nc.tensor.matmul(ph[:], wff1[:, 1, os_], x2T[:, 1, cs], start=False, stop=True)
            nc.scalar.activation(h1c[:, ot, :], ph[:], Act.Relu, bias=bff1[:, ot:ot + 1], scale=1.0)
        for tl in range(4):
            t = ch * 4 + tl
            pf = K.psA.tile([128, D], F32, tag="gemm")
            for kt in range(8):
                nc.tensor.matmul(pf[:], h1c[:, kt, tl * 128:(tl + 1) * 128], wff2[:, kt],
                                 start=(kt == 0), stop=False)
            nc.tensor.matmul(pf[:], K.ones1b[:], bf2r[:], start=False, stop=True)
            u2 = K.wkp.tile([128, D], F32, tag="u")
            nc.vector.tensor_tensor(u2[:], pf[:], x2[:, t], Alu.add)
            _ln(K, K.wkp, u2, xn[:, t], g2, b2)
    return xn


def _transpose_set_src2(K, src2, t, dst, identity, psum_tag):
    """Same as _transpose_set but src is a [128, 256] tile (no t axis)."""
    nc = K.nc
    for k in range(2):
        pt = K.psT.tile([128, 128], identity.dtype, tag=psum_tag)
        nc.tensor.transpose(pt[:], src2[:, k * 128:(k + 1) * 128], identity[:])
        nc.scalar.copy(dst[:, k, t * 128:(t + 1) * 128], pt[:])


# ---------------- device kernel builder ----------------
def _build(nlayers=NLAYERS, debug=False, bisect=()):
    import concourse.bacc as bacc
    import concourse.mybir as mybir
    import concourse.tile as tile

    dt = mybir.dt
    K = _K()
    K.Alu = mybir.AluOpType
    K.Act = mybir.ActivationFunctionType
    K.Ax = mybir.AxisListType
    K.F32, K.F32R, K.BF, K.F16, K.I32, K.I16 = (
        dt.float32, dt.float32r, dt.bfloat16, dt.float16, dt.int32, dt.int16)
    K.I8, K.U8 = dt.int8, dt.uint8

    nc = bacc.Bacc(num_devices=NCORES, num_swdge_queues=4)
    K.nc = nc
    F32, F32R, BF, F16, I32, I16 = K.F32, K.F32R, K.BF, K.F16, K.I32, K.I16

    # ---- I/O ----
    # src ships as per-row int8 (q = round(x/s), s = rowmax|x|/127) + f32 scales
    K.xq_in = nc.dram_tensor("xq", [QP, D], K.I8, kind="ExternalInput")
    K.xs_in = nc.dram_tensor("xs", [QP, 1], F32, kind="ExternalInput")
    K.posq_in = nc.dram_tensor("posq", [QP, D], F16, kind="ExternalInput")
    K.rxy_in = nc.dram_tensor("rxy", [128, T, 8], F32, kind="ExternalInput")
    K.jtab_in = nc.dram_tensor("jtab", [4, 128, NJ], I32, kind="ExternalInput")
    K.ident_in = nc.dram_tensor("ident", [128, 128], F32, kind="ExternalInput")
    K.identb_in = nc.dram_tensor("identb", [128, 128], BF, kind="ExternalInput")
    K.identh_in = nc.dram_tensor("identh", [128, 128], F16, kind="ExternalInput")
    K.ones1r_in = nc.dram_tensor("ones1r", [1, 128], F32R, kind="ExternalInput")
    K.ones1b_in = nc.dram_tensor("ones1b", [1, 128], BF, kind="ExternalInput")
    K.woa_in = nc.dram_tensor("woa", [nlayers, 2, 128, 384], F32R, kind="ExternalInput")
    K.boa_in = nc.dram_tensor("boa", [nlayers, 1, 384], F32R, kind="ExternalInput")
    K.wval_in = nc.dram_tensor("wval", [nlayers, 2, 128, D], F32R, kind="ExternalInput")
    K.bvr_in = nc.dram_tensor("bvr", [nlayers, 1, D], F32R, kind="ExternalInput")
    K.wout_in = nc.dram_tensor("wout", [nlayers, 2, 128, D], BF, kind="ExternalInput")
    K.bor_in = nc.dram_tensor("bor", [nlayers, 1, D], BF, kind="ExternalInput")
    K.wff1_in = nc.dram_tensor("wff1", [nlayers, 2, 128, DFF], F32R, kind="ExternalInput")
    K.bff1_in = nc.dram_tensor("bff1", [nlayers, 128, 8], F32, kind="ExternalInput")
    K.wff2_in = nc.dram_tensor("wff2", [nlayers, 8, 128, D], BF, kind="ExternalInput")
    K.bf2r_in = nc.dram_tensor("bf2r", [nlayers, 1, D], BF, kind="ExternalInput")
    K.lnrow_in = nc.dram_tensor("lnrow", [nlayers, 1, 1024], F32R, kind="ExternalInput")

    # output ships as per-row uint8 (u = round(x*127/amax)+128) + f32 scales
    out_t = nc.dram_tensor("out", [Q, D], K.U8, kind="ExternalOutput")
    out_s = nc.dram_tensor("out_s", [Q, 1], F32, kind="ExternalOutput")
    K.dbg = {}
    if debug:
        K.dbg["offa"] = nc.dram_tensor("dbg_offa", [128, T, 384], F32, kind="ExternalOutput")
        K.dbg["w2"] = nc.dram_tensor("dbg_w2", [128, T, 1024], BF, kind="ExternalOutput")
        K.dbg["idx"] = nc.dram_tensor("dbg_idx", [128, T, NJ], I16, kind="ExternalOutput")
        K.dbg["vf"] = nc.dram_tensor("dbg_vf", [LEN, D], BF, kind="ExternalOutput")
        K.dbg["ao"] = nc.dram_tensor("dbg_ao", [128, T, D], BF, kind="ExternalOutput")
        K.dbg["x1"] = nc.dram_tensor("dbg_x1", [128, T, D], F32, kind="ExternalOutput")

    K.groups = [[0, 1, 2, 3], [4, 5, 6, 7]]

    with tile.TileContext(nc) as tc:
        K.tc = tc
        with (
            tc.tile_pool(name="persist", bufs=1) as pp,
            tc.tile_pool(name="xstate", bufs=2) as xsp,
            tc.tile_pool(name="xtp", bufs=1) as xtp,
            tc.tile_pool(name="wlayer", bufs=1) as wlp,
            tc.tile_pool(name="brep", bufs=1) as brp,
            tc.tile_pool(name="work", bufs=3) as wkp,
            tc.tile_pool(name="wc", bufs=1) as wcp,
            tc.tile_pool(name="w2p", bufs=1) as w2p,
            tc.tile_pool(name="gather", bufs=2) as gp,
            tc.tile_pool(name="tmp", bufs=1) as tp_,
            tc.tile_pool(name="tabs", bufs=1) as tbp,
            tc.tile_pool(name="h1", bufs=1) as h1p,
            tc.tile_pool(name="psA", bufs=3, space="PSUM") as psA,
            tc.tile_pool(name="psT", bufs=2, space="PSUM") as psT,
            tc.tile_pool(name="dram", bufs=2, space="DRAM") as dram,
            tc.tile_pool(name="dramP", bufs=2, space="DRAM") as dramP,
        ):
            K.xsp, K.xtp, K.wlp, K.brp, K.wkp, K.wcp = xsp, xtp, wlp, brp, wkp, wcp
            K.w2p, K.gp, K.tp, K.tbp, K.h1p = w2p, gp, tp_, tbp, h1p
            K.psA, K.psT, K.dram, K.dramP = psA, psT, dram, dramP

            # ---------- persistent constants ----------
            K.ident = pp.tile([128, 128], F32, tag="ident")
            nc.sync.dma_start(K.ident[:], K.ident_in[:])
            K.identb = pp.tile([128, 128], BF, tag="identb")
            nc.sync.dma_start(K.identb[:], K.identb_in[:])
            K.identh = pp.tile([128, 128], F16, tag="identh")
            nc.sync.dma_start(K.identh[:], K.identh_in[:])
            K.ones1r = pp.tile([1, 128], F32R, tag="ones1r")
            nc.sync.dma_start(K.ones1r[:], K.ones1r_in[:])
            K.ones1b = pp.tile([1, 128], BF, tag="ones1b")
            nc.sync.dma_start(K.ones1b[:], K.ones1b_in[:])
            K.rxy = pp.tile([128, T, 8], F32, tag="rxy")
            nc.sync.dma_start(K.rxy[:], K.rxy_in[:])
            for i, nm in enumerate(("jW", "jWM2", "jHM2", "jLS2H")):
                tl_ = pp.tile([128, NJ], I32, tag=nm)
                nc.sync.dma_start(tl_[:], K.jtab_in[i])
                setattr(K, nm, tl_)

            # ---------- x state init (f16 -> f32) + pos^T (staged to DRAM) ----------
            x = xsp.tile([128, T, D], F32, tag="x")
            K.posT_d = dram.tile([128, 2, QP], F16, tag="posT")
            for t in range(T):
                ts = slice(t * 128, (t + 1) * 128)
                x8 = wkp.tile([128, D], K.I8, tag="io16")
                nc.sync.dma_start(x8[:], K.xq_in[ts, :])
                xsr = wkp.tile([128, 1], F32, tag="xsr")
                nc.sync.dma_start(xsr[:], K.xs_in[ts, :])
                nc.vector.tensor_copy(x[:, t], x8[:])
                nc.vector.tensor_scalar(x[:, t], x[:, t], xsr[:], None,
                                        mybir.AluOpType.mult)
                p16 = wkp.tile([128, D], F16, tag="io16")
                nc.sync.dma_start(p16[:], K.posq_in[t * 128:(t + 1) * 128, :])
                for k in range(2):
                    pt = psT.tile([128, 128], F16, tag="tpb")
                    nc.tensor.transpose(pt[:], p16[:, k * 128:(k + 1) * 128], K.identh[:])
                    ps = wkp.tile([128, 128], F16, tag="pTq")
                    nc.scalar.copy(ps[:], pt[:])
                    nc.sync.dma_start(K.posT_d[:, k, t * 128:(t + 1) * 128], ps[:])

            K.bisect = bisect
            for layer in range(nlayers):
                x = _layer(K, layer, x, debug and layer == 0)

            # ---- output (per-row int8 quant: u8 = round(x*127/amax) + 128) ----
            Alu = mybir.AluOpType
            Ax = mybir.AxisListType
            for t in range(T):
                nrows = min(128, Q - t * 128)
                mx = wkp.tile([128, 1], F32, tag="q_mx")
                mn = wkp.tile([128, 1], F32, tag="q_mn")
                nc.vector.tensor_reduce(mx[:], x[:, t], Ax.X, Alu.max)
                nc.vector.tensor_reduce(mn[:], x[:, t], Ax.X, Alu.min)
                nc.vector.tensor_scalar(mn[:], mn[:], -1.0, None, Alu.mult)
                nc.vector.tensor_tensor(mx[:], mx[:], mn[:], Alu.max)  # amax
                sc = wkp.tile([128, 1], F32, tag="q_sc")
                nc.vector.tensor_scalar(sc[:], mx[:], 1.0 / 127.0, None, Alu.mult)
                nc.sync.dma_start(out_s[t * 128:t * 128 + nrows, :], sc[:nrows, :])
                rc = wkp.tile([128, 1], F32, tag="q_rc")
                nc.vector.reciprocal(rc[:], sc[:])
                qf = wkp.tile([128, D], F32, tag="u")
                nc.vector.tensor_scalar(qf[:], x[:, t], rc[:], 128.5,
                                        Alu.mult, Alu.add)
                qu = wkp.tile([128, D], K.U8, tag="io16")
                nc.vector.tensor_copy(qu[:], qf[:])   # trunc -> round(q)+128
                nc.sync.dma_start(out_t[t * 128:t * 128 + nrows, :], qu[:nrows, :])

    nc.finalize()
    return nc


# ---------------- host-side prep ----------------
def _ref_points(valid_ratios):
    """Pixel-space base coords rx/ry per (b, q, lvl), exactly as the reference."""
    vr = np.asarray(valid_ratios, dtype=np.float32)
    refs = []
    for lvl, (Hl, Wl) in enumerate(LEVEL_SHAPES):
        ry, rx = np.meshgrid(
            np.linspace(0.5, Hl - 0.5, Hl, dtype=np.float32),
            np.linspace(0.5, Wl - 0.5, Wl, dtype=np.float32), indexing="ij")
        ry = ry.reshape(-1)[None] / (vr[:, None, lvl, 1] * Hl)
        rx = rx.reshape(-1)[None] / (vr[:, None, lvl, 0] * Wl)
        refs.append(np.stack([rx, ry], -1).astype(np.float32))
    ref = np.concatenate(refs, 1)                       # [B, LEN, 2]
    ref = ref[:, :, None] * vr[:, None]                 # [B, LEN, NL, 2]
    rxy = np.empty((B, LEN, NL, 2), np.float32)
    for lvl, (Hl, Wl) in enumerate(LEVEL_SHAPES):
        rxy[:, :, lvl, 0] = ref[:, :, lvl, 0] * np.float32(Wl) - np.float32(0.5)
        rxy[:, :, lvl, 1] = ref[:, :, lvl, 1] * np.float32(Hl) - np.float32(0.5)
    return rxy


def _jtables():
    jW = np.zeros(NJ, np.int32)
    jWM2 = np.zeros(NJ, np.int32)
    jHM2 = np.zeros(NJ, np.int32)
    jLS2H = np.zeros(NJ, np.int32)
    for h in range(NH):
        for lvl, (H, W) in enumerate(LEVEL_SHAPES):
            for p in range(NP):
                j = h * 16 + lvl * 4 + p
                jW[j] = W
                jWM2[j] = W - 2
                jHM2[j] = H - 2
                jLS2H[j] = 2 * LEVEL_START[lvl] + (h % 2)
    return np.stack([np.tile(v, (128, 1)) for v in (jW, jWM2, jHM2, jLS2H)])


def _static_arrays(inputs, nlayers=NLAYERS):
    """Per-input-name -> concatenated [8*s0, ...] array. Weight content is
    identical across cores; rxy differs (batch/quarter slice)."""
    f32 = np.float32
    w = {}
    woa = np.concatenate([np.asarray(inputs["W_off"], f32),
                          np.asarray(inputs["W_attn"], f32)], axis=2)[:nlayers]
    w["woa"] = np.ascontiguousarray(woa.reshape(nlayers, 2, 128, 384))
    w["boa"] = np.concatenate([np.asarray(inputs["b_off"], f32),
                               np.asarray(inputs["b_attn"], f32)], axis=1)[:nlayers, None, :]
    w["wval"] = np.ascontiguousarray(np.asarray(inputs["W_val"], f32)[:nlayers].reshape(nlayers, 2, 128, D))
    w["bvr"] = np.asarray(inputs["b_val"], f32)[:nlayers, None, :]
    w["wout"] = np.ascontiguousarray(
        np.asarray(inputs["W_out"], f32)[:nlayers].reshape(nlayers, 2, 128, D)).astype(BF16)
    w["bor"] = np.asarray(inputs["b_out"], f32)[:nlayers, None, :].astype(BF16)
    w["wff1"] = np.ascontiguousarray(np.asarray(inputs["W_ff1"], f32)[:nlayers].reshape(nlayers, 2, 128, DFF))
    w["bff1"] = np.ascontiguousarray(
        np.asarray(inputs["b_ff1"], f32)[:nlayers].reshape(nlayers, 8, 128).transpose(0, 2, 1))
    w["wff2"] = np.ascontiguousarray(
        np.asarray(inputs["W_ff2"], f32)[:nlayers].reshape(nlayers, 8, 128, D)).astype(BF16)
    w["bf2r"] = np.asarray(inputs["b_ff2"], f32)[:nlayers, None, :].astype(BF16)
    w["lnrow"] = np.concatenate(
        [np.asarray(inputs[k], f32)[:nlayers] for k in ("ln1_g", "ln1_b", "ln2_g", "ln2_b")],
        axis=1)[:, None, :]
    w["jtab"] = _jtables()
    w["ident"] = np.eye(128, dtype=f32)
    w["identb"] = np.eye(128, dtype=BF16)
    w["identh"] = np.eye(128, dtype=np.float16)
    w["ones1r"] = np.ones((1, 128), f32)
    w["ones1b"] = np.ones((1, 128), BF16)

    rxy = _ref_points(inputs["valid_ratios"])
    rxy_cores = []
    pos = np.asarray(inputs["pos"])
    pq = np.zeros((NCORES * QP, D), np.float16)
    for core in range(NCORES):
        b, r = core // 4, core % 4
        rxy_c = np.zeros((QP, 8), np.float32)
        rxy_c[:Q] = rxy[b, r * Q:(r + 1) * Q].reshape(Q, 8)
        rxy_cores.append(np.ascontiguousarray(rxy_c.reshape(T, 128, 8).transpose(1, 0, 2)))
        pq[core * QP: core * QP + Q] = pos[b, r * Q:(r + 1) * Q]

    out = {name: np.concatenate([arr] * NCORES, axis=0) for name, arr in w.items()}
    out["rxy"] = np.concatenate(rxy_cores, axis=0)
    out["posq"] = pq
    return out


def _dynamic_arrays(inputs):
    from concurrent.futures import ThreadPoolExecutor
    src = np.asarray(inputs["src"], np.float32)
    xq = np.zeros((NCORES * QP, D), np.int8)
    xs = np.zeros((NCORES * QP, 1), np.float32)

    def _fill(core):
        b, r = core // 4, core % 4
        blk = src[b, r * Q:(r + 1) * Q]
        amax = np.abs(blk).max(axis=1, keepdims=True)
        s = amax * np.float32(1.0 / 127.0)
        q = np.rint(blk / np.where(s == 0, 1, s))
        xq[core * QP: core * QP + Q] = q
        xs[core * QP: core * QP + Q] = s

    with ThreadPoolExecutor(NCORES) as ex:   # numpy ops release the GIL
        list(ex.map(_fill, range(NCORES)))
    return {"xq": xq, "xs": xs}


_STATIC_FP_KEYS = ("pos", "valid_ratios", "W_off", "b_off", "W_attn", "b_attn",
                   "W_val", "b_val", "W_out", "b_out", "ln1_g", "ln1_b", "W_ff1",
                   "b_ff1", "W_ff2", "b_ff2", "ln2_g", "ln2_b")


def _static_fingerprint(inputs):
    h = hashlib.blake2b(digest_size=16)
    for k in _STATIC_FP_KEYS:
        a = np.ascontiguousarray(np.asarray(inputs[k]))
        h.update(k.encode())
        h.update(str(a.shape).encode())
        h.update(memoryview(a).cast("B"))
    return h.hexdigest()


def _get_nc(nlayers=NLAYERS, debug=False):
    key = (nlayers, debug)
    if key not in _NC_CACHE:
        _NC_CACHE[key] = _build(nlayers, debug)
    return _NC_CACHE[key]


def _ensure_session():
    """Build nc + the cached jitted executable (same lowering as
    bass_utils.run_bass_kernel_spmd's axon path / bass2jax.run_bass_via_pjrt,
    hoisted out of the per-call path so it traces/compiles once)."""
    if _SESSION:
        return _SESSION
    import jax
    import jax.numpy as jnp
    from jax.sharding import Mesh, PartitionSpec, NamedSharding
    import warnings
    with warnings.catch_warnings():
        warnings.simplefilter("ignore")
        from jax.experimental.shard_map import shard_map
    from concourse import mybir
    from concourse.bass2jax import (_bass_exec_p, install_neuronx_cc_hook,
                                    partition_id_tensor)

    nc = _get_nc()
    install_neuronx_cc_hook()

    partition_name = nc.partition_id_tensor.name if nc.partition_id_tensor else None
    in_names, out_names, out_avals, zero_specs = [], [], [], []
    for alloc in nc.m.functions[0].allocations:
        if not isinstance(alloc, mybir.MemoryLocationSet):
            continue
        name = alloc.memorylocations[0].name
        if alloc.kind == "ExternalInput":
            if name != partition_name:
                in_names.append(name)
        elif alloc.kind == "ExternalOutput":
            out_names.append(name)
            shape = tuple(alloc.tensor_shape)
            dtype = mybir.dt.np(alloc.dtype)
            out_avals.append(jax.core.ShapedArray(shape, dtype))
            zero_specs.append((shape, dtype))
    n_params = len(in_names)
    n_outs = len(out_names)
    bind_names = list(in_names) + list(out_names)
    if partition_name is not None:
        bind_names.append(partition_name)
    donate = tuple(range(n_params, n_params + n_outs))

    dbg_name = nc.dbg_addr.name if nc.dbg_addr is not None else None

    def _body(*args):
        operands = list(args)
        if partition_name is not None:
            operands.append(partition_id_tensor())
        outs = _bass_exec_p.bind(
            *operands, out_avals=tuple(out_avals), in_names=tuple(bind_names),
            out_names=tuple(out_names), lowering_input_output_aliases=(),
            sim_require_finite=True, sim_require_nnan=True, nc=nc)
        return tuple(outs)

    devices = jax.devices()[:NCORES]
    mesh = Mesh(np.asarray(devices), ("core",))
    csh = NamedSharding(mesh, PartitionSpec("core"))
    in_specs = (PartitionSpec("core"),) * (n_params + n_outs)
    out_specs = (PartitionSpec("core"),) * n_outs
    sharded = jax.jit(
        shard_map(_body, mesh=mesh, in_specs=in_specs, out_specs=out_specs,
                  check_rep=False),
        donate_argnums=donate, keep_unused=True)

    def _zeros():
        return tuple(jnp.zeros((NCORES * s[0], *s[1:]), d) for s, d in zero_specs)

    zeros_fn = jax.jit(_zeros, out_shardings=(csh,) * n_outs)

    _SESSION.update(dict(
        jax=jax, nc=nc, sharded=sharded, zeros_fn=zeros_fn, csh=csh,
        in_names=in_names, out_names=out_names, dbg_name=dbg_name,
        static_fp=None, static_dev=None, prev_outs=None))
    return _SESSION


def kernel(**inputs):
    import time as _time
    st = _ensure_session()
    jax = st["jax"]

    # start the src upload first; fingerprinting overlaps with the transfer
    dyn = _dynamic_arrays(inputs)
    dyn_dev = {k: jax.device_put(v, st["csh"]) for k, v in dyn.items()}

    fp = _static_fingerprint(inputs)
    if st["static_fp"] != fp:
        stat = _static_arrays(inputs)
        if st["dbg_name"] is not None:
            stat[st["dbg_name"]] = np.zeros((NCORES, 2), np.uint32)
        st["static_dev"] = {k: jax.device_put(v, st["csh"]) for k, v in stat.items()}
        jax.block_until_ready(list(st["static_dev"].values()))
        st["static_fp"] = fp

    # donated result buffers: the kernel writes every element of its outputs,
    # so the previous call's (consumed) buffers work; zeros only on first use
    oi = st["out_names"].index("out")
    osi = st["out_names"].index("out_s")
    o = sc = None
    for attempt in range(3):
        try:
            outbufs = st["prev_outs"] if st["prev_outs"] is not None else st["zeros_fn"]()
            st["prev_outs"] = None
            args = [dyn_dev[n] if n in dyn_dev else st["static_dev"][n]
                    for n in st["in_names"]]
            outs = st["sharded"](*args, *outbufs)
            for ot in (outs[osi], outs[oi]):
                for s in ot.addressable_shards:
                    s.data.copy_to_host_async()   # overlap fetch-init with exec
            # scales in one small fetch; u8 shard-by-shard, dequantizing each
            # while later shards stream
            out = np.empty((B, LEN, D), np.float32)
            sc = np.asarray(outs[osi]).reshape(NCORES, Q, 1)
            from concurrent.futures import ThreadPoolExecutor

            def _fetch_deq(shard):
                core = shard.index[0].start // Q
                b, r = core // 4, core % 4
                u = np.asarray(shard.data)
                out[b, r * Q:(r + 1) * Q] = (u.astype(np.float32) - 128.0) * sc[core]
            with ThreadPoolExecutor(4) as ex:
                list(ex.map(_fetch_deq, outs[oi].addressable_shards))
            break
        except Exception:
            # transient NRT device state right after a process turnover —
            # back off and retry with fresh buffers
            if attempt == 2:
                raise
            _time.sleep(2.0)
            dyn_dev = {k: jax.device_put(v, st["csh"]) for k, v in dyn.items()}
    st["prev_outs"] = tuple(outs)
    return out


# revision 40
# speedup vs baseline: 1.1594x; 1.1594x over previous
"""Deformable-DETR transformer encoder (3 layers) on 8 Trainium2 NeuronCores.

Sharding: core c -> (batch b = c//4, query-quarter r = c%4). Each core
processes 2550 queries (padded to 2560) of one batch, all 8 heads.
Per layer the value projection is computed on the owned quarter and
all-gathered (groups of 4 cores) so every core can sample anywhere.

Sampling: for each (query, head, level, point) the 4 bilinear corners are
fetched with ONE dma_gather descriptor from a "quad" value table
valP4[pos] = [v(pos), v(pos+1), v(pos+W), v(pos+W+1)] (bf16, 256B rows),
then combined with hat-function weights (folding bilinear weights and the
attention softmax) on the vector engine.

Host<->device traffic is minimized (the axon tunnel moves ~70 MB/s):
 - src ships as per-row int8 + f32 scales (dequantized on device); the
   output is quantized per row on device (uint8 + f32 scales) and
   dequantized on the host, shard-by-shard while later shards stream.
 - pos ships once as f16 (device-cached); pos^T is computed on device, and
   q^T = x^T + pos^T per layer, so no host-side pos@W projection ships.
 - all GEMM biases are folded into the PSUM accumulation as rank-1
   (ones x bias_row) matmuls; LN params are broadcast on device.
 - weights are device-resident across kernel() calls (fingerprint-checked),
   and the jitted executable is cached, mirroring
   bass_utils.run_bass_kernel_spmd's axon path (run_bass_via_pjrt) minus
   the per-call retrace.
"""

import hashlib
import numpy as np
import ml_dtypes

# ---------------- problem constants (hardcoded) ----------------
LEVEL_SHAPES = ((48, 160), (24, 80), (12, 40), (6, 20))
LEN = sum(h * w for h, w in LEVEL_SHAPES)  # 10200
B, D, NH, NL, NP, DFF, NLAYERS = 2, 256, 8, 4, 4, 1024, 3
DH = D // NH  # 32
LEVEL_START = [0]
for _h, _w in LEVEL_SHAPES[:-1]:
    LEVEL_START.append(LEVEL_START[-1] + _h * _w)

NCORES = 8
Q = LEN // 4          # 2550 queries per core
QP = 2560             # padded
T = QP // 128         # 20 query tiles
VF_ROWS = LEN + LEVEL_SHAPES[-1][1] + 1   # val_full rows incl. pad (10221)
NJ = 128              # samples per query: j = (h 8, lvl 4, p 4)
BF16 = ml_dtypes.bfloat16

_NC_CACHE = {}
_SESSION = {}

# inputs that change every call; everything else (weights, pos embedding,
# valid_ratios-derived tables) is device-cached behind a content fingerprint
_DYN_NAMES = ("xq",)


class _K:
    """Holds builder state shared across helper functions."""
    pass


def _ln(K, pool, u, out_ap, g, b):
    nc, Alu, Act, Ax, F32 = K.nc, K.Alu, K.Act, K.Ax, K.F32
    m = pool.tile([128, 1], F32, tag="ln_m")
    nc.vector.tensor_reduce(m[:], u[:], Ax.X, Alu.add)
    nc.vector.tensor_scalar(m[:], m[:], 1.0 / 256.0, None, Alu.mult)
    c = pool.tile([128, 256], F32, tag="ln_c")
    nc.vector.tensor_scalar(c[:], u[:], m[:], None, Alu.subtract)
    scr = pool.tile([128, 256], F32, tag="ln_scr")
    v = pool.tile([128, 1], F32, tag="ln_v")
    nc.scalar.activation(scr[:], c[:], Act.Square, accum_out=v[:])
    nc.vector.tensor_scalar(v[:], v[:], 1.0 / 256.0, 1e-5, Alu.mult, Alu.add)
    nc.scalar.activation(v[:], v[:], Act.Sqrt)
    nc.vector.reciprocal(v[:], v[:])
    nc.vector.scalar_tensor_tensor(out_ap, c[:], v[:], g, Alu.mult, Alu.mult)
    nc.vector.tensor_tensor(out_ap, out_ap, b, Alu.add)


def _hats(K, cc, bf, h0, h1, dd):
    # h0 = relu(1-|c-b|), h1 = relu(1-|c-b-1|)
    nc, Alu = K.nc, K.Alu
    nc.vector.tensor_tensor(dd[:], cc[:], bf[:], Alu.subtract)
    nc.vector.tensor_scalar(h0[:], dd[:], -1.0, None, Alu.mult)
    nc.vector.tensor_tensor(h0[:], h0[:], dd[:], Alu.max)
    nc.vector.tensor_scalar(h0[:], h0[:], -1.0, 1.0, Alu.mult, Alu.add)
    nc.vector.tensor_scalar(h0[:], h0[:], 0.0, None, Alu.max)
    nc.vector.tensor_scalar(dd[:], dd[:], -1.0, None, Alu.add)
    nc.vector.tensor_scalar(h1[:], dd[:], -1.0, None, Alu.mult)
    nc.vector.tensor_tensor(h1[:], h1[:], dd[:], Alu.max)
    nc.vector.tensor_scalar(h1[:], h1[:], -1.0, 1.0, Alu.mult, Alu.add)
    nc.vector.tensor_scalar(h1[:], h1[:], 0.0, None, Alu.max)


def _weight_calc(K, t, offa, w2, idx_all, tl):
    """Per-sample sampling weights + gather indices for query tile t."""
    nc, Alu, Act, Ax = K.nc, K.Alu, K.Act, K.Ax
    F32, I32 = K.F32, K.I32
    wcp, rxy = K.wcp, K.rxy

    def off_ap(xy):
        return offa[:, 0:256].rearrange(
            "q (h lvl p two) -> q h lvl p two", h=8, lvl=4, p=4, two=2)[:, :, :, :, xy]

    def rxy_ap(xy):
        a = rxy[:, t, :].rearrange("q (lvl two) -> q lvl two", lvl=4)[:, :, xy]
        return a.unsqueeze(1).broadcast_to([128, 8, 4]).unsqueeze(3).broadcast_to([128, 8, 4, 4])

    jv = "q (h lvl p) -> q h lvl p"
    cx = wcp.tile([128, NJ], F32, tag="cx")
    cy = wcp.tile([128, NJ], F32, tag="cy")
    nc.vector.tensor_tensor(cx.rearrange(jv, h=8, lvl=4), off_ap(0), rxy_ap(0), Alu.add)
    nc.vector.tensor_tensor(cy.rearrange(jv, h=8, lvl=4), off_ap(1), rxy_ap(1), Alu.add)

    bxi = wcp.tile([128, NJ], I32, tag="bxi")
    byi = wcp.tile([128, NJ], I32, tag="byi")
    nc.vector.tensor_copy(bxi[:], cx[:])   # trunc cast
    nc.vector.tensor_copy(byi[:], cy[:])
    nc.vector.tensor_scalar(bxi[:], bxi[:], 0, None, Alu.max)
    nc.vector.tensor_scalar(byi[:], byi[:], 0, None, Alu.max)
    nc.vector.tensor_tensor(bxi[:], bxi[:], K.jWM2[:], Alu.min)
    nc.vector.tensor_tensor(byi[:], byi[:], K.jHM2[:], Alu.min)
    bxf = wcp.tile([128, NJ], F32, tag="bxf")
    byf = wcp.tile([128, NJ], F32, tag="byf")
    nc.vector.tensor_copy(bxf[:], bxi[:])
    nc.vector.tensor_copy(byf[:], byi[:])

    hx0 = wcp.tile([128, NJ], F32, tag="hx0")
    hx1 = wcp.tile([128, NJ], F32, tag="hx1")
    hy0 = wcp.tile([128, NJ], F32, tag="hy0")
    hy1 = wcp.tile([128, NJ], F32, tag="hy1")
    dd = wcp.tile([128, NJ], F32, tag="dd")
    _hats(K, cx, bxf, hx0, hx1, dd)
    _hats(K, cy, byf, hy0, hy1, dd)

    # attention softmax over (lvl,p) per head
    ex = wcp.tile([128, 128], F32, tag="ex")
    nc.scalar.activation(ex[:], offa[:, 256:384], Act.Exp)
    es = wcp.tile([128, 8], F32, tag="es")
    nc.vector.tensor_reduce(es[:], ex.rearrange("q (h f) -> q h f", h=8), Ax.X, Alu.add)
    er = wcp.tile([128, 8], F32, tag="er")
    nc.vector.reciprocal(er[:], es[:])
    a2 = wcp.tile([128, 128], F32, tag="a2")
    nc.vector.tensor_tensor(
        a2.rearrange("q (h f) -> q h f", h=8),
        ex.rearrange("q (h f) -> q h f", h=8),
        er.unsqueeze(2).broadcast_to([128, 8, 16]), Alu.mult)

    wy0 = wcp.tile([128, NJ], F32, tag="wy0")
    wy1 = wcp.tile([128, NJ], F32, tag="wy1")
    nc.vector.tensor_tensor(wy0[:], hy0[:], a2[:], Alu.mult)
    nc.vector.tensor_tensor(wy1[:], hy1[:], a2[:], Alu.mult)

    # w2[q, tl, j*8+s*2+dup] = wy_sy * hx_sx   (s = sy*2+sx)
    for sy, wyv in ((0, wy0), (1, wy1)):
        for sx, hxv in ((0, hx0), (1, hx1)):
            outap = w2[:, tl, :].rearrange("q (j s dup) -> q j s dup", j=NJ, s=4)[:, :, sy * 2 + sx, :]
            nc.vector.tensor_tensor(
                outap, wyv.unsqueeze(2).broadcast_to([128, NJ, 2]),
                hxv.unsqueeze(2).broadcast_to([128, NJ, 2]), Alu.mult)

    # idx = ((LS + by*W + bx) << 1) + hp   (jLS2H = 2*LS+hp)
    nc.vector.tensor_tensor(byi[:], byi[:], K.jW[:], Alu.mult)
    nc.vector.tensor_tensor(byi[:], byi[:], bxi[:], Alu.add)
    nc.vector.tensor_scalar(byi[:], byi[:], 1, None, Alu.logical_shift_left)
    nc.vector.tensor_tensor(byi[:], byi[:], K.jLS2H[:], Alu.add)
    nc.vector.tensor_copy(idx_all[:, tl], byi[:])


def _transpose_set(K, src3, t, dst, identity, psum_tag):
    """PE-transpose src3[:, t, k*128:(k+1)*128] into dst[:, k, t*128:...] for k=0,1."""
    nc = K.nc
    for k in range(2):
        pt = K.psT.tile([128, 128], identity.dtype, tag=psum_tag)
        nc.tensor.transpose(pt[:], src3[:, t, k * 128:(k + 1) * 128], identity[:])
        nc.scalar.copy(dst[:, k, t * 128:(t + 1) * 128], pt[:])


def _layer(K, layer, x, dbg_on):
    nc, Alu, Act = K.nc, K.Alu, K.Act
    F32, F32R, BF, F16, I16 = K.F32, K.F32R, K.BF, K.F16, K.I16
    dbg = K.dbg

    # ---- per-layer weights ----
    wlp, brp = K.wlp, K.brp
    woa = wlp.tile([128, 2, 384], F32R, tag="woa")
    nc.sync.dma_start(woa[:], K.woa_in[layer].rearrange("k p n -> p k n"))
    wval = wlp.tile([128, 2, D], F32R, tag="wval")
    nc.sync.dma_start(wval[:], K.wval_in[layer].rearrange("k p n -> p k n"))
    wout = wlp.tile([128, 2, D], BF, tag="wout")
    nc.sync.dma_start(wout[:], K.wout_in[layer].rearrange("k p n -> p k n"))
    wff1 = wlp.tile([128, 2, DFF], F32R, tag="wff1")
    nc.sync.dma_start(wff1[:], K.wff1_in[layer].rearrange("k p n -> p k n"))
    wff2 = wlp.tile([128, 8, D], BF, tag="wff2")
    nc.sync.dma_start(wff2[:], K.wff2_in[layer].rearrange("k p n -> p k n"))

    # bias rows (added via rank-1 ones x row matmuls inside PSUM groups)
    boa_l = brp.tile([1, 384], F32R, tag="boa")
    nc.sync.dma_start(boa_l[:], K.boa_in[layer])
    bvr = brp.tile([1, D], F32R, tag="bvr")
    nc.sync.dma_start(bvr[:], K.bvr_in[layer])
    bor = brp.tile([1, D], BF, tag="bor")
    nc.sync.dma_start(bor[:], K.bor_in[layer])
    bf2r = brp.tile([1, D], BF, tag="bf2r")
    nc.sync.dma_start(bf2r[:], K.bf2r_in[layer])
    lnr = brp.tile([1, 1024], F32R, tag="lnr")
    nc.sync.dma_start(lnr[:], K.lnrow_in[layer])
    bff1 = brp.tile([128, 8], F32, tag="bff1")
    nc.sync.dma_start(bff1[:], K.bff1_in[layer])

    # LN params broadcast to all 128 partitions: lngb = ones x (g1|b1|g2|b2)
    lngb = wlp.tile([128, 1024], F32, tag="lngb")
    for hh in range(2):
        pl = K.psA.tile([128, 512], F32, tag="gemm")
        nc.tensor.matmul(pl[:], K.ones1r[:], lnr[:, hh * 512:(hh + 1) * 512],
                         start=True, stop=True)
        nc.scalar.copy(lngb[:, hh * 512:(hh + 1) * 512], pl[:])
    g1, b1 = lngb[:, 0:256], lngb[:, 256:512]
    g2, b2 = lngb[:, 512:768], lngb[:, 768:1024]

    # ---- x^T ----
    xT = K.xtp.tile([128, 2, QP], F32R, tag="xT")
    for t in range(T):
        _transpose_set(K, x, t, xT, K.ident, "tp")

    # ---- val GEMM -> bounce -> AllGather -> valP4 ----
    vbounce = K.dram.tile([Q, D], BF, tag="vb")
    for t in range(T):
        pv = K.psA.tile([128, D], F32, tag="gemm")
        ts = slice(t * 128, (t + 1) * 128)
        nc.tensor.matmul(pv[:], xT[:, 0, ts], wval[:, 0], start=True, stop=False)
        nc.tensor.matmul(pv[:], xT[:, 1, ts], wval[:, 1], start=False, stop=False)
        nc.tensor.matmul(pv[:], K.ones1r[:], bvr[:], start=False, stop=True)
        sval = K.wkp.tile([128, D], BF, tag="sval")
        nc.scalar.copy(sval[:], pv[:])
        nrows = min(128, Q - t * 128)
        nc.sync.dma_start(vbounce[t * 128:t * 128 + nrows, :], sval[:nrows, :])
    valfull = K.dram.tile([VF_ROWS, D], BF, tag="vf")
    if "nocoll" in K.bisect:
        for rr in range(4):
            nc.sync.dma_start(valfull[rr * Q:(rr + 1) * Q, :], vbounce[:])
    else:
        nc.gpsimd.collective_compute(
            "AllGather", Alu.bypass, replica_groups=K.groups,
            ins=[vbounce[:].opt()], outs=[valfull[0:LEN, :].opt()])
    if dbg_on:
        nc.sync.dma_start(dbg["vf"][:], valfull[0:LEN, :])

    # valP4[h2][pos*2+hp] = [v(pos), v(pos+1), v(pos+W), v(pos+W+1)] of head h2*2+hp
    valP4 = [K.dramP.tile([2 * VF_ROWS, 128], BF, tag=f"vp{h2}", name=f"valP4_{h2}") for h2 in range(4)]
    for h2 in range(4):
        for lvl, (H, W) in enumerate(LEVEL_SHAPES):
            npos = H * W
            base = LEVEL_START[lvl]
            for c, dc in enumerate((0, 1, W, W + 1)):
                src = valfull[base + dc: base + dc + npos,
                              h2 * 64:(h2 + 1) * 64].rearrange("pos (hp ch) -> pos hp ch", hp=2)
                dst = valP4[h2][2 * base: 2 * (base + npos),
                                c * 32:(c + 1) * 32].rearrange("(pos hp) ch -> pos hp ch", hp=2)
                nc.sync.dma_start(dst, src)

    # ---- off/attn GEMM + weight calc + idx + table shuffle (2 halves) ----
    w2h, tabh = [], []
    for half in range(2):
        w2 = K.w2p.tile([128, 10, 1024], BF, tag="w2")
        idx_all = K.w2p.tile([128, 10, NJ], I16, tag="idx")
        for tl in range(10):
            t = half * 10 + tl
            ts = slice(t * 128, (t + 1) * 128)
            # q^T tile = x^T + pos^T (pos projection happens here on device)
            pTs = K.wkp.tile([128, 2, 128], F16, tag="pTs")
            nc.sync.dma_start(pTs[:], K.posT_d[:, :, ts])
            qTt = K.wkp.tile([128, 2, 128], F32R, tag="qTt")
            nc.vector.tensor_tensor(qTt[:], xT[:, :, ts], pTs[:], Alu.add)
            po = K.psA.tile([128, 384], F32, tag="gemm")
            nc.tensor.matmul(po[:], qTt[:, 0], woa[:, 0], start=True, stop=False)
            nc.tensor.matmul(po[:], qTt[:, 1], woa[:, 1], start=False, stop=False)
            nc.tensor.matmul(po[:], K.ones1r[:], boa_l[:], start=False, stop=True)
            offa = K.wkp.tile([128, 384], F32, tag="offa")
            nc.scalar.copy(offa[:], po[:])
            if dbg_on:
                nc.sync.dma_start(dbg["offa"][:, t], offa[:])
            _weight_calc(K, t, offa, w2, idx_all, tl)
        if dbg_on:
            nc.sync.dma_start(dbg["w2"][:, half * 10:(half + 1) * 10], w2[:])
            nc.sync.dma_start(dbg["idx"][:, half * 10:(half + 1) * 10], idx_all[:])

        for qt in range(2):
            tb = K.tbp.tile([128, 5 * 1024], I16, tag="tb", name=f"tb_{half}_{qt}")
            for qhi in range(8):
                src = idx_all[qhi * 16:(qhi + 1) * 16, qt * 5:(qt + 1) * 5, :].rearrange(
                    "q tl (h2 bb) -> q tl h2 bb", h2=4)
                dst = tb[0:16, :].rearrange("q (tl h2 bb qhi) -> q tl h2 bb qhi",
                                            tl=5, h2=4, bb=32)[:, :, :, :, qhi]
                nc.sync.dma_start(dst, src)
            for rep in range(1, 8):
                nc.sync.dma_start(tb[rep * 16:(rep + 1) * 16, :], tb[0:16, :])
            tabh.append(tb)
        w2h.append(w2)

    # ---- gather + weighting -> attn_out -> aoT ----
    aoT = K.xtp.tile([128, 2, QP], BF, tag="aoT")
    for t in range(T):
        ao = K.wkp.tile([128, D], BF, tag="ao")
        tb, w2, tl = tabh[t // 5], w2h[t // 10], t % 10
        tq = t % 5
        for h2 in range(4):
            G = K.gp.tile([128, 32, 128], BF, tag="G")
            if "nogather" in K.bisect:
                nc.gpsimd.memset(G[:], 0.25)
            else:
                for q4 in range(4):
                    co = tq * 1024 + h2 * 256 + q4 * 64
                    nc.gpsimd.dma_gather(
                        G[:, q4 * 8:(q4 + 1) * 8, :], valP4[h2][:],
                        tb[:, co: co + 64],
                        num_idxs=1024, num_idxs_reg=1024, elem_size=128,
                        queue_num=q4)
            tmp = K.tp.tile([128, 4096], BF, tag="tmp")
            g_ap = G[:].rearrange("q b e -> q (b e)").rearrange("q (g ch) -> q g ch", ch=32)
            w_ap = w2[:, tl, h2 * 256:(h2 + 1) * 256].rearrange(
                "q (g dup) -> q g dup", dup=2).unsqueeze(2).broadcast_to([128, 128, 16, 2])
            nc.vector.tensor_tensor(tmp.rearrange("q (g ch) -> q g ch", ch=32), g_ap, w_ap, Alu.mult)
            # tree reduce over (lvl, p, s) keeping (hp, ch); layout (hp 2, lvl 4, p 4, s 4, ch 32)
            cur, n = tmp, 2048
            for _ in range(6):
                nxt = K.tp.tile([128, n], BF, tag=f"r{n}")
                va = cur.rearrange("q (hp f) -> q hp f", hp=2)
                nc.vector.tensor_tensor(
                    nxt.rearrange("q (hp f) -> q hp f", hp=2),
                    va[:, :, 0:n // 2], va[:, :, n // 2:n], Alu.add)
                cur, n = nxt, n // 2
            nc.vector.tensor_copy(ao[:, h2 * 64:(h2 + 1) * 64], cur[:])
        if dbg_on:
            nc.sync.dma_start(dbg["ao"][:, t], ao[:])
        _transpose_set_src2(K, ao, t, aoT, K.identb, "tpb")

    # ---- out proj + residual + LN1 ----
    x2 = K.xsp.tile([128, T, D], F32, tag="x")
    for t in range(T):
        po = K.psA.tile([128, D], F32, tag="gemm")
        ts = slice(t * 128, (t + 1) * 128)
        nc.tensor.matmul(po[:], aoT[:, 0, ts], wout[:, 0], start=True, stop=False)
        nc.tensor.matmul(po[:], aoT[:, 1, ts], wout[:, 1], start=False, stop=False)
        nc.tensor.matmul(po[:], K.ones1b[:], bor[:], start=False, stop=True)
        u = K.wkp.tile([128, D], F32, tag="u")
        nc.vector.tensor_tensor(u[:], po[:], x[:, t], Alu.add)
        _ln(K, K.wkp, u, x2[:, t], g1, b1)
        if dbg_on:
            nc.sync.dma_start(dbg["x1"][:, t], x2[:, t])

    # ---- FFN (chunked over 512 queries) ----
    x2T = K.xtp.tile([128, 2, QP], F32R, tag="xT")
    for t in range(T):
        _transpose_set(K, x2, t, x2T, K.ident, "tp")
    xn = K.xsp.tile([128, T, D], F32, tag="x")
    for ch in range(5):
        h1c = K.h1p.tile([128, 8, 512], BF, tag="h1c")
        cs = slice(ch * 512, (ch + 1) * 512)
        for ot in range(8):
            ph = K.psA.tile([128, 512], F32, tag="gemm")
            os_ = slice(ot * 128, (ot + 1) * 128)
            nc.tensor.matmul(ph[:], wff1[:, 0, os_], x2T[:, 0, cs], start=True, stop=False)
            nc.tensor.matmul(ph[:], wff1[:, 1, os_], x2T[:, 1, cs], start=False, stop=True)
            nc.scalar.activation(h1c[:, ot, :], ph[:], Act.Relu, bias=bff1[:, ot:ot + 1], scale=1.0)
        for tl in range(4):
            t = ch * 4 + tl
            pf = K.psA.tile([128, D], F32, tag="gemm")
            for kt in range(8):
                nc.tensor.matmul(pf[:], h1c[:, kt, tl * 128:(tl + 1) * 128], wff2[:, kt],
                                 start=(kt == 0), stop=False)
            nc.tensor.matmul(pf[:], K.ones1b[:], bf2r[:], start=False, stop=True)
            u2 = K.wkp.tile([128, D], F32, tag="u")
            nc.vector.tensor_tensor(u2[:], pf[:], x2[:, t], Alu.add)
            _ln(K, K.wkp, u2, xn[:, t], g2, b2)
    return xn


def _transpose_set_src2(K, src2, t, dst, identity, psum_tag):
    """Same as _transpose_set but src is a [128, 256] tile (no t axis)."""
    nc = K.nc
    for k in range(2):
        pt = K.psT.tile([128, 128], identity.dtype, tag=psum_tag)
        nc.tensor.transpose(pt[:], src2[:, k * 128:(k + 1) * 128], identity[:])
        nc.scalar.copy(dst[:, k, t * 128:(t + 1) * 128], pt[:])


# ---------------- device kernel builder ----------------
def _build(nlayers=NLAYERS, debug=False, bisect=()):
    import concourse.bacc as bacc
    import concourse.mybir as mybir
    import concourse.tile as tile

    dt = mybir.dt
    K = _K()
    K.Alu = mybir.AluOpType
    K.Act = mybir.ActivationFunctionType
    K.Ax = mybir.AxisListType
    K.F32, K.F32R, K.BF, K.F16, K.I32, K.I16 = (
        dt.float32, dt.float32r, dt.bfloat16, dt.float16, dt.int32, dt.int16)
    K.I8, K.U8 = dt.int8, dt.uint8

    nc = bacc.Bacc(num_devices=NCORES, num_swdge_queues=4)
    K.nc = nc
    F32, F32R, BF, F16, I32, I16 = K.F32, K.F32R, K.BF, K.F16, K.I32, K.I16

    # ---- I/O ----
    # src ships as per-row int8 (q = round(x/s), s = rowmax|x|/127) + f32 scales
    K.xq_in = nc.dram_tensor("xq", [QP, D], K.I8, kind="ExternalInput")
    K.xs_in = nc.dram_tensor("xs", [QP, 1], F32, kind="ExternalInput")
    K.posq_in = nc.dram_tensor("posq", [QP, D], F16, kind="ExternalInput")
    K.rxy_in = nc.dram_tensor("rxy", [128, T, 8], F32, kind="ExternalInput")
    K.jtab_in = nc.dram_tensor("jtab", [4, 128, NJ], I32, kind="ExternalInput")
    K.ident_in = nc.dram_tensor("ident", [128, 128], F32, kind="ExternalInput")
    K.identb_in = nc.dram_tensor("identb", [128, 128], BF, kind="ExternalInput")
    K.identh_in = nc.dram_tensor("identh", [128, 128], F16, kind="ExternalInput")
    K.ones1r_in = nc.dram_tensor("ones1r", [1, 128], F32R, kind="ExternalInput")
    K.ones1b_in = nc.dram_tensor("ones1b", [1, 128], BF, kind="ExternalInput")
    K.woa_in = nc.dram_tensor("woa", [nlayers, 2, 128, 384], F32R, kind="ExternalInput")
    K.boa_in = nc.dram_tensor("boa", [nlayers, 1, 384], F32R, kind="ExternalInput")
    K.wval_in = nc.dram_tensor("wval", [nlayers, 2, 128, D], F32R, kind="ExternalInput")
    K.bvr_in = nc.dram_tensor("bvr", [nlayers, 1, D], F32R, kind="ExternalInput")
    K.wout_in = nc.dram_tensor("wout", [nlayers, 2, 128, D], BF, kind="ExternalInput")
    K.bor_in = nc.dram_tensor("bor", [nlayers, 1, D], BF, kind="ExternalInput")
    K.wff1_in = nc.dram_tensor("wff1", [nlayers, 2, 128, DFF], F32R, kind="ExternalInput")
    K.bff1_in = nc.dram_tensor("bff1", [nlayers, 128, 8], F32, kind="ExternalInput")
    K.wff2_in = nc.dram_tensor("wff2", [nlayers, 8, 128, D], BF, kind="ExternalInput")
    K.bf2r_in = nc.dram_tensor("bf2r", [nlayers, 1, D], BF, kind="ExternalInput")
    K.lnrow_in = nc.dram_tensor("lnrow", [nlayers, 1, 1024], F32R, kind="ExternalInput")

    # output ships as per-row uint8 (u = round(x*127/amax)+128) + f32 scales
    out_t = nc.dram_tensor("out", [Q, D], K.U8, kind="ExternalOutput")
    out_s = nc.dram_tensor("out_s", [Q, 1], F32, kind="ExternalOutput")
    K.dbg = {}
    if debug:
        K.dbg["offa"] = nc.dram_tensor("dbg_offa", [128, T, 384], F32, kind="ExternalOutput")
        K.dbg["w2"] = nc.dram_tensor("dbg_w2", [128, T, 1024], BF, kind="ExternalOutput")
        K.dbg["idx"] = nc.dram_tensor("dbg_idx", [128, T, NJ], I16, kind="ExternalOutput")
        K.dbg["vf"] = nc.dram_tensor("dbg_vf", [LEN, D], BF, kind="ExternalOutput")
        K.dbg["ao"] = nc.dram_tensor("dbg_ao", [128, T, D], BF, kind="ExternalOutput")
        K.dbg["x1"] = nc.dram_tensor("dbg_x1", [128, T, D], F32, kind="ExternalOutput")

    K.groups = [[0, 1, 2, 3], [4, 5, 6, 7]]

    with tile.TileContext(nc) as tc:
        K.tc = tc
        with (
            tc.tile_pool(name="persist", bufs=1) as pp,
            tc.tile_pool(name="xstate", bufs=2) as xsp,
            tc.tile_pool(name="xtp", bufs=1) as xtp,
            tc.tile_pool(name="wlayer", bufs=1) as wlp,
            tc.tile_pool(name="brep", bufs=1) as brp,
            tc.tile_pool(name="work", bufs=3) as wkp,
            tc.tile_pool(name="wc", bufs=1) as wcp,
            tc.tile_pool(name="w2p", bufs=1) as w2p,
            tc.tile_pool(name="gather", bufs=2) as gp,
            tc.tile_pool(name="tmp", bufs=1) as tp_,
            tc.tile_pool(name="tabs", bufs=1) as tbp,
            tc.tile_pool(name="h1", bufs=1) as h1p,
            tc.tile_pool(name="psA", bufs=3, space="PSUM") as psA,
            tc.tile_pool(name="psT", bufs=2, space="PSUM") as psT,
            tc.tile_pool(name="dram", bufs=2, space="DRAM") as dram,
            tc.tile_pool(name="dramP", bufs=2, space="DRAM") as dramP,
        ):
            K.xsp, K.xtp, K.wlp, K.brp, K.wkp, K.wcp = xsp, xtp, wlp, brp, wkp, wcp
            K.w2p, K.gp, K.tp, K.tbp, K.h1p = w2p, gp, tp_, tbp, h1p
            K.psA, K.psT, K.dram, K.dramP = psA, psT, dram, dramP

            # ---------- persistent constants ----------
            K.ident = pp.tile([128, 128], F32, tag="ident")
            nc.sync.dma_start(K.ident[:], K.ident_in[:])
            K.identb = pp.tile([128, 128], BF, tag="identb")
            nc.sync.dma_start(K.identb[:], K.identb_in[:])
            K.identh = pp.tile([128, 128], F16, tag="identh")
            nc.sync.dma_start(K.identh[:], K.identh_in[:])
            K.ones1r = pp.tile([1, 128], F32R, tag="ones1r")
            nc.sync.dma_start(K.ones1r[:], K.ones1r_in[:])
            K.ones1b = pp.tile([1, 128], BF, tag="ones1b")
            nc.sync.dma_start(K.ones1b[:], K.ones1b_in[:])
            K.rxy = pp.tile([128, T, 8], F32, tag="rxy")
            nc.sync.dma_start(K.rxy[:], K.rxy_in[:])
            for i, nm in enumerate(("jW", "jWM2", "jHM2", "jLS2H")):
                tl_ = pp.tile([128, NJ], I32, tag=nm)
                nc.sync.dma_start(tl_[:], K.jtab_in[i])
                setattr(K, nm, tl_)

            # ---------- x state init (f16 -> f32) + pos^T (staged to DRAM) ----------
            x = xsp.tile([128, T, D], F32, tag="x")
            K.posT_d = dram.tile([128, 2, QP], F16, tag="posT")
            for t in range(T):
                ts = slice(t * 128, (t + 1) * 128)
                x8 = wkp.tile([128, D], K.I8, tag="io16")
                nc.sync.dma_start(x8[:], K.xq_in[ts, :])
                xsr = wkp.tile([128, 1], F32, tag="xsr")
                nc.sync.dma_start(xsr[:], K.xs_in[ts, :])
                nc.vector.tensor_copy(x[:, t], x8[:])
                nc.vector.tensor_scalar(x[:, t], x[:, t], xsr[:], None,
                                        mybir.AluOpType.mult)
                p16 = wkp.tile([128, D], F16, tag="io16")
                nc.sync.dma_start(p16[:], K.posq_in[t * 128:(t + 1) * 128, :])
                for k in range(2):
                    pt = psT.tile([128, 128], F16, tag="tpb")
                    nc.tensor.transpose(pt[:], p16[:, k * 128:(k + 1) * 128], K.identh[:])
                    ps = wkp.tile([128, 128], F16, tag="pTq")
                    nc.scalar.copy(ps[:], pt[:])
                    nc.sync.dma_start(K.posT_d[:, k, t * 128:(t + 1) * 128], ps[:])

            K.bisect = bisect
            for layer in range(nlayers):
                x = _layer(K, layer, x, debug and layer == 0)

            # ---- output (per-row int8 quant: u8 = round(x*127/amax) + 128) ----
            Alu = mybir.AluOpType
            Ax = mybir.AxisListType
            for t in range(T):
                nrows = min(128, Q - t * 128)
                mx = wkp.tile([128, 1], F32, tag="q_mx")
                mn = wkp.tile([128, 1], F32, tag="q_mn")
                nc.vector.tensor_reduce(mx[:], x[:, t], Ax.X, Alu.max)
                nc.vector.tensor_reduce(mn[:], x[:, t], Ax.X, Alu.min)
                nc.vector.tensor_scalar(mn[:], mn[:], -1.0, None, Alu.mult)
                nc.vector.tensor_tensor(mx[:], mx[:], mn[:], Alu.max)  # amax
                sc = wkp.tile([128, 1], F32, tag="q_sc")
                nc.vector.tensor_scalar(sc[:], mx[:], 1.0 / 127.0, None, Alu.mult)
                nc.sync.dma_start(out_s[t * 128:t * 128 + nrows, :], sc[:nrows, :])
                rc = wkp.tile([128, 1], F32, tag="q_rc")
                nc.vector.reciprocal(rc[:], sc[:])
                qf = wkp.tile([128, D], F32, tag="u")
                nc.vector.tensor_scalar(qf[:], x[:, t], rc[:], 128.5,
                                        Alu.mult, Alu.add)
                qu = wkp.tile([128, D], K.U8, tag="io16")
                nc.vector.tensor_copy(qu[:], qf[:])   # trunc -> round(q)+128
                nc.sync.dma_start(out_t[t * 128:t * 128 + nrows, :], qu[:nrows, :])

    nc.finalize()
    return nc


# ---------------- host-side prep ----------------
def _ref_points(valid_ratios):
    """Pixel-space base coords rx/ry per (b, q, lvl), exactly as the reference."""
    vr = np.asarray(valid_ratios, dtype=np.float32)
    refs = []
    for lvl, (Hl, Wl) in enumerate(LEVEL_SHAPES):
        ry, rx = np.meshgrid(
            np.linspace(0.5, Hl - 0.5, Hl, dtype=np.float32),
            np.linspace(0.5, Wl - 0.5, Wl, dtype=np.float32), indexing="ij")
        ry = ry.reshape(-1)[None] / (vr[:, None, lvl, 1] * Hl)
        rx = rx.reshape(-1)[None] / (vr[:, None, lvl, 0] * Wl)
        refs.append(np.stack([rx, ry], -1).astype(np.float32))
    ref = np.concatenate(refs, 1)                       # [B, LEN, 2]
    ref = ref[:, :, None] * vr[:, None]                 # [B, LEN, NL, 2]
    rxy = np.empty((B, LEN, NL, 2), np.float32)
    for lvl, (Hl, Wl) in enumerate(LEVEL_SHAPES):
        rxy[:, :, lvl, 0] = ref[:, :, lvl, 0] * np.float32(Wl) - np.float32(0.5)
        rxy[:, :, lvl, 1] = ref[:, :, lvl, 1] * np.float32(Hl) - np.float32(0.5)
    return rxy


def _jtables():
    jW = np.zeros(NJ, np.int32)
    jWM2 = np.zeros(NJ, np.int32)
    jHM2 = np.zeros(NJ, np.int32)
    jLS2H = np.zeros(NJ, np.int32)
    for h in range(NH):
        for lvl, (H, W) in enumerate(LEVEL_SHAPES):
            for p in range(NP):
                j = h * 16 + lvl * 4 + p
                jW[j] = W
                jWM2[j] = W - 2
                jHM2[j] = H - 2
                jLS2H[j] = 2 * LEVEL_START[lvl] + (h % 2)
    return np.stack([np.tile(v, (128, 1)) for v in (jW, jWM2, jHM2, jLS2H)])


def _static_arrays(inputs, nlayers=NLAYERS):
    """Per-input-name -> concatenated [8*s0, ...] array. Weight content is
    identical across cores; rxy differs (batch/quarter slice)."""
    f32 = np.float32
    w = {}
    woa = np.concatenate([np.asarray(inputs["W_off"], f32),
                          np.asarray(inputs["W_attn"], f32)], axis=2)[:nlayers]
    w["woa"] = np.ascontiguousarray(woa.reshape(nlayers, 2, 128, 384))
    w["boa"] = np.concatenate([np.asarray(inputs["b_off"], f32),
                               np.asarray(inputs["b_attn"], f32)], axis=1)[:nlayers, None, :]
    w["wval"] = np.ascontiguousarray(np.asarray(inputs["W_val"], f32)[:nlayers].reshape(nlayers, 2, 128, D))
    w["bvr"] = np.asarray(inputs["b_val"], f32)[:nlayers, None, :]
    w["wout"] = np.ascontiguousarray(
        np.asarray(inputs["W_out"], f32)[:nlayers].reshape(nlayers, 2, 128, D)).astype(BF16)
    w["bor"] = np.asarray(inputs["b_out"], f32)[:nlayers, None, :].astype(BF16)
    w["wff1"] = np.ascontiguousarray(np.asarray(inputs["W_ff1"], f32)[:nlayers].reshape(nlayers, 2, 128, DFF))
    w["bff1"] = np.ascontiguousarray(
        np.asarray(inputs["b_ff1"], f32)[:nlayers].reshape(nlayers, 8, 128).transpose(0, 2, 1))
    w["wff2"] = np.ascontiguousarray(
        np.asarray(inputs["W_ff2"], f32)[:nlayers].reshape(nlayers, 8, 128, D)).astype(BF16)
    w["bf2r"] = np.asarray(inputs["b_ff2"], f32)[:nlayers, None, :].astype(BF16)
    w["lnrow"] = np.concatenate(
        [np.asarray(inputs[k], f32)[:nlayers] for k in ("ln1_g", "ln1_b", "ln2_g", "ln2_b")],
        axis=1)[:, None, :]
    w["jtab"] = _jtables()
    w["ident"] = np.eye(128, dtype=f32)
    w["identb"] = np.eye(128, dtype=BF16)
    w["identh"] = np.eye(128, dtype=np.float16)
    w["ones1r"] = np.ones((1, 128), f32)
    w["ones1b"] = np.ones((1, 128), BF16)

    rxy = _ref_points(inputs["valid_ratios"])
    rxy_cores = []
    pos = np.asarray(inputs["pos"])
    pq = np.zeros((NCORES * QP, D), np.float16)
    for core in range(NCORES):
        b, r = core // 4, core % 4
        rxy_c = np.zeros((QP, 8), np.float32)
        rxy_c[:Q] = rxy[b, r * Q:(r + 1) * Q].reshape(Q, 8)
        rxy_cores.append(np.ascontiguousarray(rxy_c.reshape(T, 128, 8).transpose(1, 0, 2)))
        pq[core * QP: core * QP + Q] = pos[b, r * Q:(r + 1) * Q]

    out = {name: np.concatenate([arr] * NCORES, axis=0) for name, arr in w.items()}
    out["rxy"] = np.concatenate(rxy_cores, axis=0)
    out["posq"] = pq
    return out


def _dynamic_arrays(inputs):
    from concurrent.futures import ThreadPoolExecutor
    src = np.asarray(inputs["src"], np.float32)
    xq = np.zeros((NCORES * QP, D), np.int8)
    xs = np.zeros((NCORES * QP, 1), np.float32)

    def _fill(core):
        b, r = core // 4, core % 4
        blk = src[b, r * Q:(r + 1) * Q]
        amax = np.abs(blk).max(axis=1, keepdims=True)
        s = amax * np.float32(1.0 / 127.0)
        q = np.rint(blk / np.where(s == 0, 1, s))
        xq[core * QP: core * QP + Q] = q
        xs[core * QP: core * QP + Q] = s

    with ThreadPoolExecutor(NCORES) as ex:   # numpy ops release the GIL
        list(ex.map(_fill, range(NCORES)))
    return {"xq": xq, "xs": xs}


_STATIC_FP_KEYS = ("pos", "valid_ratios", "W_off", "b_off", "W_attn", "b_attn",
                   "W_val", "b_val", "W_out", "b_out", "ln1_g", "ln1_b", "W_ff1",
                   "b_ff1", "W_ff2", "b_ff2", "ln2_g", "ln2_b")


def _static_fingerprint(inputs):
    h = hashlib.blake2b(digest_size=16)
    for k in _STATIC_FP_KEYS:
        a = np.ascontiguousarray(np.asarray(inputs[k]))
        h.update(k.encode())
        h.update(str(a.shape).encode())
        h.update(memoryview(a).cast("B"))
    return h.hexdigest()


def _get_nc(nlayers=NLAYERS, debug=False):
    key = (nlayers, debug)
    if key not in _NC_CACHE:
        _NC_CACHE[key] = _build(nlayers, debug)
    return _NC_CACHE[key]


def _ensure_session():
    """Build nc + the cached jitted executable (same lowering as
    bass_utils.run_bass_kernel_spmd's axon path / bass2jax.run_bass_via_pjrt,
    hoisted out of the per-call path so it traces/compiles once)."""
    if _SESSION:
        return _SESSION
    import jax
    import jax.numpy as jnp
    from jax.sharding import Mesh, PartitionSpec, NamedSharding
    import warnings
    with warnings.catch_warnings():
        warnings.simplefilter("ignore")
        from jax.experimental.shard_map import shard_map
    from concourse import mybir
    from concourse.bass2jax import (_bass_exec_p, install_neuronx_cc_hook,
                                    partition_id_tensor)

    nc = _get_nc()
    install_neuronx_cc_hook()

    partition_name = nc.partition_id_tensor.name if nc.partition_id_tensor else None
    in_names, out_names, out_avals, zero_specs = [], [], [], []
    for alloc in nc.m.functions[0].allocations:
        if not isinstance(alloc, mybir.MemoryLocationSet):
            continue
        name = alloc.memorylocations[0].name
        if alloc.kind == "ExternalInput":
            if name != partition_name:
                in_names.append(name)
        elif alloc.kind == "ExternalOutput":
            out_names.append(name)
            shape = tuple(alloc.tensor_shape)
            dtype = mybir.dt.np(alloc.dtype)
            out_avals.append(jax.core.ShapedArray(shape, dtype))
            zero_specs.append((shape, dtype))
    n_params = len(in_names)
    n_outs = len(out_names)
    bind_names = list(in_names) + list(out_names)
    if partition_name is not None:
        bind_names.append(partition_name)
    donate = tuple(range(n_params, n_params + n_outs))

    dbg_name = nc.dbg_addr.name if nc.dbg_addr is not None else None

    def _body(*args):
        operands = list(args)
        if partition_name is not None:
            operands.append(partition_id_tensor())
        outs = _bass_exec_p.bind(
            *operands, out_avals=tuple(out_avals), in_names=tuple(bind_names),
            out_names=tuple(out_names), lowering_input_output_aliases=(),
            sim_require_finite=True, sim_require_nnan=True, nc=nc)
        return tuple(outs)

    devices = jax.devices()[:NCORES]
    mesh = Mesh(np.asarray(devices), ("core",))
    csh = NamedSharding(mesh, PartitionSpec("core"))
    in_specs = (PartitionSpec("core"),) * (n_params + n_outs)
    out_specs = (PartitionSpec("core"),) * n_outs
    sharded = jax.jit(
        shard_map(_body, mesh=mesh, in_specs=in_specs, out_specs=out_specs,
                  check_rep=False),
        donate_argnums=donate, keep_unused=True)

    def _zeros():
        return tuple(jnp.zeros((NCORES * s[0], *s[1:]), d) for s, d in zero_specs)

    zeros_fn = jax.jit(_zeros, out_shardings=(csh,) * n_outs)

    _SESSION.update(dict(
        jax=jax, nc=nc, sharded=sharded, zeros_fn=zeros_fn, csh=csh,
        in_names=in_names, out_names=out_names, dbg_name=dbg_name,
        static_fp=None, static_dev=None, prev_outs=None))
    return _SESSION


def _refresh_static(st, inputs, fp):
    jax = st["jax"]
    stat = _static_arrays(inputs)
    if st["dbg_name"] is not None:
        stat[st["dbg_name"]] = np.zeros((NCORES, 2), np.uint32)
    st["static_dev"] = {k: jax.device_put(v, st["csh"]) for k, v in stat.items()}
    jax.block_until_ready(list(st["static_dev"].values()))
    st["static_fp"] = fp


def kernel(**inputs):
    import time as _time
    from concurrent.futures import ThreadPoolExecutor
    st = _ensure_session()
    jax = st["jax"]

    # fingerprint in a worker so dispatch doesn't wait on it (verified below,
    # before any result is returned); src upload starts right after quant
    if "fpex" not in st:
        st["fpex"] = ThreadPoolExecutor(1)
    fp_fut = st["fpex"].submit(_static_fingerprint, inputs)
    dyn = _dynamic_arrays(inputs)
    dyn_dev = {k: jax.device_put(v, st["csh"]) for k, v in dyn.items()}

    if st["static_fp"] is None:
        _refresh_static(st, inputs, fp_fut.result())
        fp_fut = None   # already consumed; statics known-fresh

    # donated result buffers: the kernel writes every element of its outputs,
    # so the previous call's (consumed) buffers work; zeros only on first use
    oi = st["out_names"].index("out")
    osi = st["out_names"].index("out_s")
    o = sc = None
    for attempt in range(3):
        try:
            outbufs = st["prev_outs"] if st["prev_outs"] is not None else st["zeros_fn"]()
            st["prev_outs"] = None
            args = [dyn_dev[n] if n in dyn_dev else st["static_dev"][n]
                    for n in st["in_names"]]
            outs = st["sharded"](*args, *outbufs)
            if fp_fut is not None:
                # optimistic dispatch used cached statics — verify now, while
                # the execute round-trip is in flight
                fp = fp_fut.result()
                fp_fut = None
                if fp != st["static_fp"]:
                    # statics changed: refresh and re-execute before returning
                    _refresh_static(st, inputs, fp)
                    args = [dyn_dev[n] if n in dyn_dev else st["static_dev"][n]
                            for n in st["in_names"]]
                    outs = st["sharded"](*args, *outs)
            for ot in (outs[osi], outs[oi]):
                for s in ot.addressable_shards:
                    s.data.copy_to_host_async()   # overlap fetch-init with exec
            # scales in one small fetch; u8 shard-by-shard, dequantizing each
            # while later shards stream
            out = np.empty((B, LEN, D), np.float32)
            sc = np.asarray(outs[osi]).reshape(NCORES, Q, 1)
            from concurrent.futures import ThreadPoolExecutor

            def _fetch_deq(shard):
                core = shard.index[0].start // Q
                b, r = core // 4, core % 4
                u = np.asarray(shard.data)
                out[b, r * Q:(r + 1) * Q] = (u.astype(np.float32) - 128.0) * sc[core]
            with ThreadPoolExecutor(4) as ex:
                list(ex.map(_fetch_deq, outs[oi].addressable_shards))
            break
        except Exception:
            # transient NRT device state right after a process turnover —
            # back off and retry with fresh buffers
            if attempt == 2:
                raise
            _time.sleep(2.0)
            dyn_dev = {k: jax.device_put(v, st["csh"]) for k, v in dyn.items()}
    st["prev_outs"] = tuple(outs)
    return out


# revision 42
# speedup vs baseline: 1.1600x; 1.0005x over previous
"""Deformable-DETR transformer encoder (3 layers) on 8 Trainium2 NeuronCores.

Sharding: core c -> (batch b = c//4, query-quarter r = c%4). Each core
processes 2550 queries (padded to 2560) of one batch, all 8 heads.
Per layer the value projection is computed on the owned quarter and
all-gathered (groups of 4 cores) so every core can sample anywhere.

Sampling: for each (query, head, level, point) the 4 bilinear corners are
fetched with ONE dma_gather descriptor from a "quad" value table
valP4[pos] = [v(pos), v(pos+1), v(pos+W), v(pos+W+1)] (bf16, 256B rows),
then combined with hat-function weights (folding bilinear weights and the
attention softmax) on the vector engine.

Host<->device traffic is minimized (the axon tunnel moves ~70 MB/s):
 - src ships as per-row int8 + f32 scales (dequantized on device); the
   output is quantized per row on device (uint8 + f32 scales) and
   dequantized on the host, shard-by-shard while later shards stream.
 - pos ships once as f16 (device-cached); pos^T is computed on device, and
   q^T = x^T + pos^T per layer, so no host-side pos@W projection ships.
 - all GEMM biases are folded into the PSUM accumulation as rank-1
   (ones x bias_row) matmuls; LN params are broadcast on device.
 - weights are device-resident across kernel() calls (fingerprint-checked),
   and the jitted executable is cached, mirroring
   bass_utils.run_bass_kernel_spmd's axon path (run_bass_via_pjrt) minus
   the per-call retrace.
"""

import hashlib
import numpy as np
import ml_dtypes

# ---------------- problem constants (hardcoded) ----------------
LEVEL_SHAPES = ((48, 160), (24, 80), (12, 40), (6, 20))
LEN = sum(h * w for h, w in LEVEL_SHAPES)  # 10200
B, D, NH, NL, NP, DFF, NLAYERS = 2, 256, 8, 4, 4, 1024, 3
DH = D // NH  # 32
LEVEL_START = [0]
for _h, _w in LEVEL_SHAPES[:-1]:
    LEVEL_START.append(LEVEL_START[-1] + _h * _w)

NCORES = 8
Q = LEN // 4          # 2550 queries per core
QP = 2560             # padded
T = QP // 128         # 20 query tiles
VF_ROWS = LEN + LEVEL_SHAPES[-1][1] + 1   # val_full rows incl. pad (10221)
NJ = 128              # samples per query: j = (h 8, lvl 4, p 4)
BF16 = ml_dtypes.bfloat16

_NC_CACHE = {}
_SESSION = {}

# inputs that change every call; everything else (weights, pos embedding,
# valid_ratios-derived tables) is device-cached behind a content fingerprint
_DYN_NAMES = ("xq",)


class _K:
    """Holds builder state shared across helper functions."""
    pass


def _ln(K, pool, u, out_ap, g, b):
    nc, Alu, Act, Ax, F32 = K.nc, K.Alu, K.Act, K.Ax, K.F32
    m = pool.tile([128, 1], F32, tag="ln_m")
    nc.vector.tensor_reduce(m[:], u[:], Ax.X, Alu.add)
    nc.vector.tensor_scalar(m[:], m[:], 1.0 / 256.0, None, Alu.mult)
    c = pool.tile([128, 256], F32, tag="ln_c")
    nc.vector.tensor_scalar(c[:], u[:], m[:], None, Alu.subtract)
    scr = pool.tile([128, 256], F32, tag="ln_scr")
    v = pool.tile([128, 1], F32, tag="ln_v")
    nc.scalar.activation(scr[:], c[:], Act.Square, accum_out=v[:])
    nc.vector.tensor_scalar(v[:], v[:], 1.0 / 256.0, 1e-5, Alu.mult, Alu.add)
    nc.scalar.activation(v[:], v[:], Act.Sqrt)
    nc.vector.reciprocal(v[:], v[:])
    nc.vector.scalar_tensor_tensor(out_ap, c[:], v[:], g, Alu.mult, Alu.mult)
    nc.vector.tensor_tensor(out_ap, out_ap, b, Alu.add)


def _hats(K, cc, bf, h0, h1, dd):
    # h0 = relu(1-|c-b|), h1 = relu(1-|c-b-1|)
    nc, Alu = K.nc, K.Alu
    nc.vector.tensor_tensor(dd[:], cc[:], bf[:], Alu.subtract)
    nc.vector.tensor_scalar(h0[:], dd[:], -1.0, None, Alu.mult)
    nc.vector.tensor_tensor(h0[:], h0[:], dd[:], Alu.max)
    nc.vector.tensor_scalar(h0[:], h0[:], -1.0, 1.0, Alu.mult, Alu.add)
    nc.vector.tensor_scalar(h0[:], h0[:], 0.0, None, Alu.max)
    nc.vector.tensor_scalar(dd[:], dd[:], -1.0, None, Alu.add)
    nc.vector.tensor_scalar(h1[:], dd[:], -1.0, None, Alu.mult)
    nc.vector.tensor_tensor(h1[:], h1[:], dd[:], Alu.max)
    nc.vector.tensor_scalar(h1[:], h1[:], -1.0, 1.0, Alu.mult, Alu.add)
    nc.vector.tensor_scalar(h1[:], h1[:], 0.0, None, Alu.max)


def _weight_calc(K, t, offa, w2, idx_all, tl):
    """Per-sample sampling weights + gather indices for query tile t."""
    nc, Alu, Act, Ax = K.nc, K.Alu, K.Act, K.Ax
    F32, I32 = K.F32, K.I32
    wcp, rxy = K.wcp, K.rxy

    def off_ap(xy):
        return offa[:, 0:256].rearrange(
            "q (h lvl p two) -> q h lvl p two", h=8, lvl=4, p=4, two=2)[:, :, :, :, xy]

    def rxy_ap(xy):
        a = rxy[:, t, :].rearrange("q (lvl two) -> q lvl two", lvl=4)[:, :, xy]
        return a.unsqueeze(1).broadcast_to([128, 8, 4]).unsqueeze(3).broadcast_to([128, 8, 4, 4])

    jv = "q (h lvl p) -> q h lvl p"
    cx = wcp.tile([128, NJ], F32, tag="cx")
    cy = wcp.tile([128, NJ], F32, tag="cy")
    nc.vector.tensor_tensor(cx.rearrange(jv, h=8, lvl=4), off_ap(0), rxy_ap(0), Alu.add)
    nc.vector.tensor_tensor(cy.rearrange(jv, h=8, lvl=4), off_ap(1), rxy_ap(1), Alu.add)

    bxi = wcp.tile([128, NJ], I32, tag="bxi")
    byi = wcp.tile([128, NJ], I32, tag="byi")
    nc.vector.tensor_copy(bxi[:], cx[:])   # trunc cast
    nc.vector.tensor_copy(byi[:], cy[:])
    nc.vector.tensor_scalar(bxi[:], bxi[:], 0, None, Alu.max)
    nc.vector.tensor_scalar(byi[:], byi[:], 0, None, Alu.max)
    nc.vector.tensor_tensor(bxi[:], bxi[:], K.jWM2[:], Alu.min)
    nc.vector.tensor_tensor(byi[:], byi[:], K.jHM2[:], Alu.min)
    bxf = wcp.tile([128, NJ], F32, tag="bxf")
    byf = wcp.tile([128, NJ], F32, tag="byf")
    nc.vector.tensor_copy(bxf[:], bxi[:])
    nc.vector.tensor_copy(byf[:], byi[:])

    hx0 = wcp.tile([128, NJ], F32, tag="hx0")
    hx1 = wcp.tile([128, NJ], F32, tag="hx1")
    hy0 = wcp.tile([128, NJ], F32, tag="hy0")
    hy1 = wcp.tile([128, NJ], F32, tag="hy1")
    dd = wcp.tile([128, NJ], F32, tag="dd")
    _hats(K, cx, bxf, hx0, hx1, dd)
    _hats(K, cy, byf, hy0, hy1, dd)

    # attention softmax over (lvl,p) per head
    ex = wcp.tile([128, 128], F32, tag="ex")
    nc.scalar.activation(ex[:], offa[:, 256:384], Act.Exp)
    es = wcp.tile([128, 8], F32, tag="es")
    nc.vector.tensor_reduce(es[:], ex.rearrange("q (h f) -> q h f", h=8), Ax.X, Alu.add)
    er = wcp.tile([128, 8], F32, tag="er")
    nc.vector.reciprocal(er[:], es[:])
    a2 = wcp.tile([128, 128], F32, tag="a2")
    nc.vector.tensor_tensor(
        a2.rearrange("q (h f) -> q h f", h=8),
        ex.rearrange("q (h f) -> q h f", h=8),
        er.unsqueeze(2).broadcast_to([128, 8, 16]), Alu.mult)

    wy0 = wcp.tile([128, NJ], F32, tag="wy0")
    wy1 = wcp.tile([128, NJ], F32, tag="wy1")
    nc.vector.tensor_tensor(wy0[:], hy0[:], a2[:], Alu.mult)
    nc.vector.tensor_tensor(wy1[:], hy1[:], a2[:], Alu.mult)

    # w2[q, tl, j*8+s*2+dup] = wy_sy * hx_sx   (s = sy*2+sx)
    for sy, wyv in ((0, wy0), (1, wy1)):
        for sx, hxv in ((0, hx0), (1, hx1)):
            outap = w2[:, tl, :].rearrange("q (j s dup) -> q j s dup", j=NJ, s=4)[:, :, sy * 2 + sx, :]
            nc.vector.tensor_tensor(
                outap, wyv.unsqueeze(2).broadcast_to([128, NJ, 2]),
                hxv.unsqueeze(2).broadcast_to([128, NJ, 2]), Alu.mult)

    # idx = ((LS + by*W + bx) << 1) + hp   (jLS2H = 2*LS+hp)
    nc.vector.tensor_tensor(byi[:], byi[:], K.jW[:], Alu.mult)
    nc.vector.tensor_tensor(byi[:], byi[:], bxi[:], Alu.add)
    nc.vector.tensor_scalar(byi[:], byi[:], 1, None, Alu.logical_shift_left)
    nc.vector.tensor_tensor(byi[:], byi[:], K.jLS2H[:], Alu.add)
    nc.vector.tensor_copy(idx_all[:, tl], byi[:])


def _transpose_set(K, src3, t, dst, identity, psum_tag):
    """PE-transpose src3[:, t, k*128:(k+1)*128] into dst[:, k, t*128:...] for k=0,1."""
    nc = K.nc
    for k in range(2):
        pt = K.psT.tile([128, 128], identity.dtype, tag=psum_tag)
        nc.tensor.transpose(pt[:], src3[:, t, k * 128:(k + 1) * 128], identity[:])
        nc.scalar.copy(dst[:, k, t * 128:(t + 1) * 128], pt[:])


def _layer(K, layer, x, dbg_on):
    nc, Alu, Act = K.nc, K.Alu, K.Act
    F32, F32R, BF, F16, I16 = K.F32, K.F32R, K.BF, K.F16, K.I16
    dbg = K.dbg

    # ---- per-layer weights ----
    wlp, brp = K.wlp, K.brp
    woa = wlp.tile([128, 2, 384], F32R, tag="woa")
    nc.sync.dma_start(woa[:], K.woa_in[layer].rearrange("k p n -> p k n"))
    wval = wlp.tile([128, 2, D], F32R, tag="wval")
    nc.sync.dma_start(wval[:], K.wval_in[layer].rearrange("k p n -> p k n"))
    wout = wlp.tile([128, 2, D], BF, tag="wout")
    nc.sync.dma_start(wout[:], K.wout_in[layer].rearrange("k p n -> p k n"))
    wff1 = wlp.tile([128, 2, DFF], F32R, tag="wff1")
    nc.sync.dma_start(wff1[:], K.wff1_in[layer].rearrange("k p n -> p k n"))
    wff2 = wlp.tile([128, 8, D], BF, tag="wff2")
    nc.sync.dma_start(wff2[:], K.wff2_in[layer].rearrange("k p n -> p k n"))

    # bias rows (added via rank-1 ones x row matmuls inside PSUM groups)
    boa_l = brp.tile([1, 384], F32R, tag="boa")
    nc.sync.dma_start(boa_l[:], K.boa_in[layer])
    bvr = brp.tile([1, D], F32R, tag="bvr")
    nc.sync.dma_start(bvr[:], K.bvr_in[layer])
    bor = brp.tile([1, D], BF, tag="bor")
    nc.sync.dma_start(bor[:], K.bor_in[layer])
    bf2r = brp.tile([1, D], BF, tag="bf2r")
    nc.sync.dma_start(bf2r[:], K.bf2r_in[layer])
    lnr = brp.tile([1, 1024], F32R, tag="lnr")
    nc.sync.dma_start(lnr[:], K.lnrow_in[layer])
    bff1 = brp.tile([128, 8], F32, tag="bff1")
    nc.sync.dma_start(bff1[:], K.bff1_in[layer])

    # LN params broadcast to all 128 partitions: lngb = ones x (g1|b1|g2|b2)
    lngb = wlp.tile([128, 1024], F32, tag="lngb")
    for hh in range(2):
        pl = K.psA.tile([128, 512], F32, tag="gemm")
        nc.tensor.matmul(pl[:], K.ones1r[:], lnr[:, hh * 512:(hh + 1) * 512],
                         start=True, stop=True)
        nc.scalar.copy(lngb[:, hh * 512:(hh + 1) * 512], pl[:])
    g1, b1 = lngb[:, 0:256], lngb[:, 256:512]
    g2, b2 = lngb[:, 512:768], lngb[:, 768:1024]

    # ---- x^T ----
    xT = K.xtp.tile([128, 2, QP], F32R, tag="xT")
    for t in range(T):
        _transpose_set(K, x, t, xT, K.ident, "tp")

    # ---- val GEMM -> bounce -> AllGather -> valP4 ----
    vbounce = K.dram.tile([Q, D], BF, tag="vb")
    for t in range(T):
        pv = K.psA.tile([128, D], F32, tag="gemm")
        ts = slice(t * 128, (t + 1) * 128)
        nc.tensor.matmul(pv[:], xT[:, 0, ts], wval[:, 0], start=True, stop=False)
        nc.tensor.matmul(pv[:], xT[:, 1, ts], wval[:, 1], start=False, stop=False)
        nc.tensor.matmul(pv[:], K.ones1r[:], bvr[:], start=False, stop=True)
        sval = K.wkp.tile([128, D], BF, tag="sval")
        nc.scalar.copy(sval[:], pv[:])
        nrows = min(128, Q - t * 128)
        nc.sync.dma_start(vbounce[t * 128:t * 128 + nrows, :], sval[:nrows, :])
    valfull = K.dram.tile([VF_ROWS, D], BF, tag="vf")
    if "nocoll" in K.bisect:
        for rr in range(4):
            nc.sync.dma_start(valfull[rr * Q:(rr + 1) * Q, :], vbounce[:])
    else:
        nc.gpsimd.collective_compute(
            "AllGather", Alu.bypass, replica_groups=K.groups,
            ins=[vbounce[:].opt()], outs=[valfull[0:LEN, :].opt()])
    if dbg_on:
        nc.sync.dma_start(dbg["vf"][:], valfull[0:LEN, :])

    # valP4[h2][pos*2+hp] = [v(pos), v(pos+1), v(pos+W), v(pos+W+1)] of head h2*2+hp
    valP4 = [K.dramP.tile([2 * VF_ROWS, 128], BF, tag=f"vp{h2}", name=f"valP4_{h2}") for h2 in range(4)]
    for h2 in range(4):
        for lvl, (H, W) in enumerate(LEVEL_SHAPES):
            npos = H * W
            base = LEVEL_START[lvl]
            for c, dc in enumerate((0, 1, W, W + 1)):
                src = valfull[base + dc: base + dc + npos,
                              h2 * 64:(h2 + 1) * 64].rearrange("pos (hp ch) -> pos hp ch", hp=2)
                dst = valP4[h2][2 * base: 2 * (base + npos),
                                c * 32:(c + 1) * 32].rearrange("(pos hp) ch -> pos hp ch", hp=2)
                nc.sync.dma_start(dst, src)

    # ---- off/attn GEMM + weight calc + idx + table shuffle (2 halves) ----
    w2h, tabh = [], []
    for half in range(2):
        w2 = K.w2p.tile([128, 10, 1024], BF, tag="w2")
        idx_all = K.w2p.tile([128, 10, NJ], I16, tag="idx")
        for tl in range(10):
            t = half * 10 + tl
            ts = slice(t * 128, (t + 1) * 128)
            # q^T tile = x^T + pos^T (pos projection happens here on device)
            pTs = K.wkp.tile([128, 2, 128], F16, tag="pTs")
            nc.sync.dma_start(pTs[:], K.posT_d[:, :, ts])
            qTt = K.wkp.tile([128, 2, 128], F32R, tag="qTt")
            nc.vector.tensor_tensor(qTt[:], xT[:, :, ts], pTs[:], Alu.add)
            po = K.psA.tile([128, 384], F32, tag="gemm")
            nc.tensor.matmul(po[:], qTt[:, 0], woa[:, 0], start=True, stop=False)
            nc.tensor.matmul(po[:], qTt[:, 1], woa[:, 1], start=False, stop=False)
            nc.tensor.matmul(po[:], K.ones1r[:], boa_l[:], start=False, stop=True)
            offa = K.wkp.tile([128, 384], F32, tag="offa")
            nc.scalar.copy(offa[:], po[:])
            if dbg_on:
                nc.sync.dma_start(dbg["offa"][:, t], offa[:])
            _weight_calc(K, t, offa, w2, idx_all, tl)
        if dbg_on:
            nc.sync.dma_start(dbg["w2"][:, half * 10:(half + 1) * 10], w2[:])
            nc.sync.dma_start(dbg["idx"][:, half * 10:(half + 1) * 10], idx_all[:])

        for qt in range(2):
            tb = K.tbp.tile([128, 5 * 1024], I16, tag="tb", name=f"tb_{half}_{qt}")
            for qhi in range(8):
                src = idx_all[qhi * 16:(qhi + 1) * 16, qt * 5:(qt + 1) * 5, :].rearrange(
                    "q tl (h2 bb) -> q tl h2 bb", h2=4)
                dst = tb[0:16, :].rearrange("q (tl h2 bb qhi) -> q tl h2 bb qhi",
                                            tl=5, h2=4, bb=32)[:, :, :, :, qhi]
                nc.sync.dma_start(dst, src)
            for rep in range(1, 8):
                nc.sync.dma_start(tb[rep * 16:(rep + 1) * 16, :], tb[0:16, :])
            tabh.append(tb)
        w2h.append(w2)

    # ---- gather + weighting -> attn_out -> aoT ----
    aoT = K.xtp.tile([128, 2, QP], BF, tag="aoT")
    for t in range(T):
        ao = K.wkp.tile([128, D], BF, tag="ao")
        tb, w2, tl = tabh[t // 5], w2h[t // 10], t % 10
        tq = t % 5
        for h2 in range(4):
            G = K.gp.tile([128, 32, 128], BF, tag="G")
            if "nogather" in K.bisect:
                nc.gpsimd.memset(G[:], 0.25)
            else:
                for q4 in range(4):
                    co = tq * 1024 + h2 * 256 + q4 * 64
                    nc.gpsimd.dma_gather(
                        G[:, q4 * 8:(q4 + 1) * 8, :], valP4[h2][:],
                        tb[:, co: co + 64],
                        num_idxs=1024, num_idxs_reg=1024, elem_size=128,
                        queue_num=q4)
            tmp = K.tp.tile([128, 4096], BF, tag="tmp")
            g_ap = G[:].rearrange("q b e -> q (b e)").rearrange("q (g ch) -> q g ch", ch=32)
            w_ap = w2[:, tl, h2 * 256:(h2 + 1) * 256].rearrange(
                "q (g dup) -> q g dup", dup=2).unsqueeze(2).broadcast_to([128, 128, 16, 2])
            nc.vector.tensor_tensor(tmp.rearrange("q (g ch) -> q g ch", ch=32), g_ap, w_ap, Alu.mult)
            # tree reduce over (lvl, p, s) keeping (hp, ch); layout (hp 2, lvl 4, p 4, s 4, ch 32)
            cur, n = tmp, 2048
            for _ in range(6):
                nxt = K.tp.tile([128, n], BF, tag=f"r{n}")
                va = cur.rearrange("q (hp f) -> q hp f", hp=2)
                nc.vector.tensor_tensor(
                    nxt.rearrange("q (hp f) -> q hp f", hp=2),
                    va[:, :, 0:n // 2], va[:, :, n // 2:n], Alu.add)
                cur, n = nxt, n // 2
            nc.vector.tensor_copy(ao[:, h2 * 64:(h2 + 1) * 64], cur[:])
        if dbg_on:
            nc.sync.dma_start(dbg["ao"][:, t], ao[:])
        _transpose_set_src2(K, ao, t, aoT, K.identb, "tpb")

    # ---- out proj + residual + LN1 ----
    x2 = K.xsp.tile([128, T, D], F32, tag="x")
    for t in range(T):
        po = K.psA.tile([128, D], F32, tag="gemm")
        ts = slice(t * 128, (t + 1) * 128)
        nc.tensor.matmul(po[:], aoT[:, 0, ts], wout[:, 0], start=True, stop=False)
        nc.tensor.matmul(po[:], aoT[:, 1, ts], wout[:, 1], start=False, stop=False)
        nc.tensor.matmul(po[:], K.ones1b[:], bor[:], start=False, stop=True)
        u = K.wkp.tile([128, D], F32, tag="u")
        nc.vector.tensor_tensor(u[:], po[:], x[:, t], Alu.add)
        _ln(K, K.wkp, u, x2[:, t], g1, b1)
        if dbg_on:
            nc.sync.dma_start(dbg["x1"][:, t], x2[:, t])

    # ---- FFN (chunked over 512 queries) ----
    x2T = K.xtp.tile([128, 2, QP], F32R, tag="xT")
    for t in range(T):
        _transpose_set(K, x2, t, x2T, K.ident, "tp")
    xn = K.xsp.tile([128, T, D], F32, tag="x")
    for ch in range(5):
        h1c = K.h1p.tile([128, 8, 512], BF, tag="h1c")
        cs = slice(ch * 512, (ch + 1) * 512)
        for ot in range(8):
            ph = K.psA.tile([128, 512], F32, tag="gemm")
            os_ = slice(ot * 128, (ot + 1) * 128)
            nc.tensor.matmul(ph[:], wff1[:, 0, os_], x2T[:, 0, cs], start=True, stop=False)
            nc.tensor.matmul(ph[:], wff1[:, 1, os_], x2T[:, 1, cs], start=False, stop=True)
            nc.scalar.activation(h1c[:, ot, :], ph[:], Act.Relu, bias=bff1[:, ot:ot + 1], scale=1.0)
        for tl in range(4):
            t = ch * 4 + tl
            pf = K.psA.tile([128, D], F32, tag="gemm")
            for kt in range(8):
                nc.tensor.matmul(pf[:], h1c[:, kt, tl * 128:(tl + 1) * 128], wff2[:, kt],
                                 start=(kt == 0), stop=False)
            nc.tensor.matmul(pf[:], K.ones1b[:], bf2r[:], start=False, stop=True)
            u2 = K.wkp.tile([128, D], F32, tag="u")
            nc.vector.tensor_tensor(u2[:], pf[:], x2[:, t], Alu.add)
            _ln(K, K.wkp, u2, xn[:, t], g2, b2)
    return xn


def _transpose_set_src2(K, src2, t, dst, identity, psum_tag):
    """Same as _transpose_set but src is a [128, 256] tile (no t axis)."""
    nc = K.nc
    for k in range(2):
        pt = K.psT.tile([128, 128], identity.dtype, tag=psum_tag)
        nc.tensor.transpose(pt[:], src2[:, k * 128:(k + 1) * 128], identity[:])
        nc.scalar.copy(dst[:, k, t * 128:(t + 1) * 128], pt[:])


# ---------------- device kernel builder ----------------
def _build(nlayers=NLAYERS, debug=False, bisect=()):
    import concourse.bacc as bacc
    import concourse.mybir as mybir
    import concourse.tile as tile

    dt = mybir.dt
    K = _K()
    K.Alu = mybir.AluOpType
    K.Act = mybir.ActivationFunctionType
    K.Ax = mybir.AxisListType
    K.F32, K.F32R, K.BF, K.F16, K.I32, K.I16 = (
        dt.float32, dt.float32r, dt.bfloat16, dt.float16, dt.int32, dt.int16)
    K.I8, K.U8 = dt.int8, dt.uint8

    nc = bacc.Bacc(num_devices=NCORES, num_swdge_queues=4)
    K.nc = nc
    F32, F32R, BF, F16, I32, I16 = K.F32, K.F32R, K.BF, K.F16, K.I32, K.I16

    # ---- I/O ----
    # src ships as per-row int8 (q = round(x/s), s = rowmax|x|/127) + f32 scales
    K.xq_in = nc.dram_tensor("xq", [QP, D], K.I8, kind="ExternalInput")
    K.xs_in = nc.dram_tensor("xs", [QP, 1], F32, kind="ExternalInput")
    K.posq_in = nc.dram_tensor("posq", [QP, D], F16, kind="ExternalInput")
    K.rxy_in = nc.dram_tensor("rxy", [128, T, 8], F32, kind="ExternalInput")
    K.jtab_in = nc.dram_tensor("jtab", [4, 128, NJ], I32, kind="ExternalInput")
    K.ident_in = nc.dram_tensor("ident", [128, 128], F32, kind="ExternalInput")
    K.identb_in = nc.dram_tensor("identb", [128, 128], BF, kind="ExternalInput")
    K.identh_in = nc.dram_tensor("identh", [128, 128], F16, kind="ExternalInput")
    K.ones1r_in = nc.dram_tensor("ones1r", [1, 128], F32R, kind="ExternalInput")
    K.ones1b_in = nc.dram_tensor("ones1b", [1, 128], BF, kind="ExternalInput")
    K.woa_in = nc.dram_tensor("woa", [nlayers, 2, 128, 384], F32R, kind="ExternalInput")
    K.boa_in = nc.dram_tensor("boa", [nlayers, 1, 384], F32R, kind="ExternalInput")
    K.wval_in = nc.dram_tensor("wval", [nlayers, 2, 128, D], F32R, kind="ExternalInput")
    K.bvr_in = nc.dram_tensor("bvr", [nlayers, 1, D], F32R, kind="ExternalInput")
    K.wout_in = nc.dram_tensor("wout", [nlayers, 2, 128, D], BF, kind="ExternalInput")
    K.bor_in = nc.dram_tensor("bor", [nlayers, 1, D], BF, kind="ExternalInput")
    K.wff1_in = nc.dram_tensor("wff1", [nlayers, 2, 128, DFF], F32R, kind="ExternalInput")
    K.bff1_in = nc.dram_tensor("bff1", [nlayers, 128, 8], F32, kind="ExternalInput")
    K.wff2_in = nc.dram_tensor("wff2", [nlayers, 8, 128, D], BF, kind="ExternalInput")
    K.bf2r_in = nc.dram_tensor("bf2r", [nlayers, 1, D], BF, kind="ExternalInput")
    K.lnrow_in = nc.dram_tensor("lnrow", [nlayers, 1, 1024], F32R, kind="ExternalInput")

    # output ships as per-row uint8 (u = round(x*127/amax)+128) + f32 scales
    out_t = nc.dram_tensor("out", [Q, D], K.U8, kind="ExternalOutput")
    out_s = nc.dram_tensor("out_s", [Q, 1], F32, kind="ExternalOutput")
    K.dbg = {}
    if debug:
        K.dbg["offa"] = nc.dram_tensor("dbg_offa", [128, T, 384], F32, kind="ExternalOutput")
        K.dbg["w2"] = nc.dram_tensor("dbg_w2", [128, T, 1024], BF, kind="ExternalOutput")
        K.dbg["idx"] = nc.dram_tensor("dbg_idx", [128, T, NJ], I16, kind="ExternalOutput")
        K.dbg["vf"] = nc.dram_tensor("dbg_vf", [LEN, D], BF, kind="ExternalOutput")
        K.dbg["ao"] = nc.dram_tensor("dbg_ao", [128, T, D], BF, kind="ExternalOutput")
        K.dbg["x1"] = nc.dram_tensor("dbg_x1", [128, T, D], F32, kind="ExternalOutput")

    K.groups = [[0, 1, 2, 3], [4, 5, 6, 7]]

    with tile.TileContext(nc) as tc:
        K.tc = tc
        with (
            tc.tile_pool(name="persist", bufs=1) as pp,
            tc.tile_pool(name="xstate", bufs=2) as xsp,
            tc.tile_pool(name="xtp", bufs=1) as xtp,
            tc.tile_pool(name="wlayer", bufs=1) as wlp,
            tc.tile_pool(name="brep", bufs=1) as brp,
            tc.tile_pool(name="work", bufs=3) as wkp,
            tc.tile_pool(name="wc", bufs=1) as wcp,
            tc.tile_pool(name="w2p", bufs=1) as w2p,
            tc.tile_pool(name="gather", bufs=2) as gp,
            tc.tile_pool(name="tmp", bufs=1) as tp_,
            tc.tile_pool(name="tabs", bufs=1) as tbp,
            tc.tile_pool(name="h1", bufs=1) as h1p,
            tc.tile_pool(name="psA", bufs=3, space="PSUM") as psA,
            tc.tile_pool(name="psT", bufs=2, space="PSUM") as psT,
            tc.tile_pool(name="dram", bufs=2, space="DRAM") as dram,
            tc.tile_pool(name="dramP", bufs=2, space="DRAM") as dramP,
        ):
            K.xsp, K.xtp, K.wlp, K.brp, K.wkp, K.wcp = xsp, xtp, wlp, brp, wkp, wcp
            K.w2p, K.gp, K.tp, K.tbp, K.h1p = w2p, gp, tp_, tbp, h1p
            K.psA, K.psT, K.dram, K.dramP = psA, psT, dram, dramP

            # ---------- persistent constants ----------
            K.ident = pp.tile([128, 128], F32, tag="ident")
            nc.sync.dma_start(K.ident[:], K.ident_in[:])
            K.identb = pp.tile([128, 128], BF, tag="identb")
            nc.sync.dma_start(K.identb[:], K.identb_in[:])
            K.identh = pp.tile([128, 128], F16, tag="identh")
            nc.sync.dma_start(K.identh[:], K.identh_in[:])
            K.ones1r = pp.tile([1, 128], F32R, tag="ones1r")
            nc.sync.dma_start(K.ones1r[:], K.ones1r_in[:])
            K.ones1b = pp.tile([1, 128], BF, tag="ones1b")
            nc.sync.dma_start(K.ones1b[:], K.ones1b_in[:])
            K.rxy = pp.tile([128, T, 8], F32, tag="rxy")
            nc.sync.dma_start(K.rxy[:], K.rxy_in[:])
            for i, nm in enumerate(("jW", "jWM2", "jHM2", "jLS2H")):
                tl_ = pp.tile([128, NJ], I32, tag=nm)
                nc.sync.dma_start(tl_[:], K.jtab_in[i])
                setattr(K, nm, tl_)

            # ---------- x state init (f16 -> f32) + pos^T (staged to DRAM) ----------
            x = xsp.tile([128, T, D], F32, tag="x")
            K.posT_d = dram.tile([128, 2, QP], F16, tag="posT")
            for t in range(T):
                ts = slice(t * 128, (t + 1) * 128)
                x8 = wkp.tile([128, D], K.I8, tag="io16")
                nc.sync.dma_start(x8[:], K.xq_in[ts, :])
                xsr = wkp.tile([128, 1], F32, tag="xsr")
                nc.sync.dma_start(xsr[:], K.xs_in[ts, :])
                nc.vector.tensor_copy(x[:, t], x8[:])
                nc.vector.tensor_scalar(x[:, t], x[:, t], xsr[:], None,
                                        mybir.AluOpType.mult)
                p16 = wkp.tile([128, D], F16, tag="io16")
                nc.sync.dma_start(p16[:], K.posq_in[t * 128:(t + 1) * 128, :])
                for k in range(2):
                    pt = psT.tile([128, 128], F16, tag="tpb")
                    nc.tensor.transpose(pt[:], p16[:, k * 128:(k + 1) * 128], K.identh[:])
                    ps = wkp.tile([128, 128], F16, tag="pTq")
                    nc.scalar.copy(ps[:], pt[:])
                    nc.sync.dma_start(K.posT_d[:, k, t * 128:(t + 1) * 128], ps[:])

            K.bisect = bisect
            for layer in range(nlayers):
                x = _layer(K, layer, x, debug and layer == 0)

            # ---- output (per-row int8 quant: u8 = round(x*127/amax) + 128) ----
            Alu = mybir.AluOpType
            Ax = mybir.AxisListType
            for t in range(T):
                nrows = min(128, Q - t * 128)
                mx = wkp.tile([128, 1], F32, tag="q_mx")
                mn = wkp.tile([128, 1], F32, tag="q_mn")
                nc.vector.tensor_reduce(mx[:], x[:, t], Ax.X, Alu.max)
                nc.vector.tensor_reduce(mn[:], x[:, t], Ax.X, Alu.min)
                nc.vector.tensor_scalar(mn[:], mn[:], -1.0, None, Alu.mult)
                nc.vector.tensor_tensor(mx[:], mx[:], mn[:], Alu.max)  # amax
                sc = wkp.tile([128, 1], F32, tag="q_sc")
                nc.vector.tensor_scalar(sc[:], mx[:], 1.0 / 127.0, None, Alu.mult)
                nc.sync.dma_start(out_s[t * 128:t * 128 + nrows, :], sc[:nrows, :])
                rc = wkp.tile([128, 1], F32, tag="q_rc")
                nc.vector.reciprocal(rc[:], sc[:])
                qf = wkp.tile([128, D], F32, tag="u")
                nc.vector.tensor_scalar(qf[:], x[:, t], rc[:], 128.5,
                                        Alu.mult, Alu.add)
                qu = wkp.tile([128, D], K.U8, tag="io16")
                nc.vector.tensor_copy(qu[:], qf[:])   # trunc -> round(q)+128
                nc.sync.dma_start(out_t[t * 128:t * 128 + nrows, :], qu[:nrows, :])

    nc.finalize()
    return nc


# ---------------- host-side prep ----------------
def _ref_points(valid_ratios):
    """Pixel-space base coords rx/ry per (b, q, lvl), exactly as the reference."""
    vr = np.asarray(valid_ratios, dtype=np.float32)
    refs = []
    for lvl, (Hl, Wl) in enumerate(LEVEL_SHAPES):
        ry, rx = np.meshgrid(
            np.linspace(0.5, Hl - 0.5, Hl, dtype=np.float32),
            np.linspace(0.5, Wl - 0.5, Wl, dtype=np.float32), indexing="ij")
        ry = ry.reshape(-1)[None] / (vr[:, None, lvl, 1] * Hl)
        rx = rx.reshape(-1)[None] / (vr[:, None, lvl, 0] * Wl)
        refs.append(np.stack([rx, ry], -1).astype(np.float32))
    ref = np.concatenate(refs, 1)                       # [B, LEN, 2]
    ref = ref[:, :, None] * vr[:, None]                 # [B, LEN, NL, 2]
    rxy = np.empty((B, LEN, NL, 2), np.float32)
    for lvl, (Hl, Wl) in enumerate(LEVEL_SHAPES):
        rxy[:, :, lvl, 0] = ref[:, :, lvl, 0] * np.float32(Wl) - np.float32(0.5)
        rxy[:, :, lvl, 1] = ref[:, :, lvl, 1] * np.float32(Hl) - np.float32(0.5)
    return rxy


def _jtables():
    jW = np.zeros(NJ, np.int32)
    jWM2 = np.zeros(NJ, np.int32)
    jHM2 = np.zeros(NJ, np.int32)
    jLS2H = np.zeros(NJ, np.int32)
    for h in range(NH):
        for lvl, (H, W) in enumerate(LEVEL_SHAPES):
            for p in range(NP):
                j = h * 16 + lvl * 4 + p
                jW[j] = W
                jWM2[j] = W - 2
                jHM2[j] = H - 2
                jLS2H[j] = 2 * LEVEL_START[lvl] + (h % 2)
    return np.stack([np.tile(v, (128, 1)) for v in (jW, jWM2, jHM2, jLS2H)])


def _static_arrays(inputs, nlayers=NLAYERS):
    """Per-input-name -> concatenated [8*s0, ...] array. Weight content is
    identical across cores; rxy differs (batch/quarter slice)."""
    f32 = np.float32
    w = {}
    woa = np.concatenate([np.asarray(inputs["W_off"], f32),
                          np.asarray(inputs["W_attn"], f32)], axis=2)[:nlayers]
    w["woa"] = np.ascontiguousarray(woa.reshape(nlayers, 2, 128, 384))
    w["boa"] = np.concatenate([np.asarray(inputs["b_off"], f32),
                               np.asarray(inputs["b_attn"], f32)], axis=1)[:nlayers, None, :]
    w["wval"] = np.ascontiguousarray(np.asarray(inputs["W_val"], f32)[:nlayers].reshape(nlayers, 2, 128, D))
    w["bvr"] = np.asarray(inputs["b_val"], f32)[:nlayers, None, :]
    w["wout"] = np.ascontiguousarray(
        np.asarray(inputs["W_out"], f32)[:nlayers].reshape(nlayers, 2, 128, D)).astype(BF16)
    w["bor"] = np.asarray(inputs["b_out"], f32)[:nlayers, None, :].astype(BF16)
    w["wff1"] = np.ascontiguousarray(np.asarray(inputs["W_ff1"], f32)[:nlayers].reshape(nlayers, 2, 128, DFF))
    w["bff1"] = np.ascontiguousarray(
        np.asarray(inputs["b_ff1"], f32)[:nlayers].reshape(nlayers, 8, 128).transpose(0, 2, 1))
    w["wff2"] = np.ascontiguousarray(
        np.asarray(inputs["W_ff2"], f32)[:nlayers].reshape(nlayers, 8, 128, D)).astype(BF16)
    w["bf2r"] = np.asarray(inputs["b_ff2"], f32)[:nlayers, None, :].astype(BF16)
    w["lnrow"] = np.concatenate(
        [np.asarray(inputs[k], f32)[:nlayers] for k in ("ln1_g", "ln1_b", "ln2_g", "ln2_b")],
        axis=1)[:, None, :]
    w["jtab"] = _jtables()
    w["ident"] = np.eye(128, dtype=f32)
    w["identb"] = np.eye(128, dtype=BF16)
    w["identh"] = np.eye(128, dtype=np.float16)
    w["ones1r"] = np.ones((1, 128), f32)
    w["ones1b"] = np.ones((1, 128), BF16)

    rxy = _ref_points(inputs["valid_ratios"])
    rxy_cores = []
    pos = np.asarray(inputs["pos"])
    pq = np.zeros((NCORES * QP, D), np.float16)
    for core in range(NCORES):
        b, r = core // 4, core % 4
        rxy_c = np.zeros((QP, 8), np.float32)
        rxy_c[:Q] = rxy[b, r * Q:(r + 1) * Q].reshape(Q, 8)
        rxy_cores.append(np.ascontiguousarray(rxy_c.reshape(T, 128, 8).transpose(1, 0, 2)))
        pq[core * QP: core * QP + Q] = pos[b, r * Q:(r + 1) * Q]

    out = {name: np.concatenate([arr] * NCORES, axis=0) for name, arr in w.items()}
    out["rxy"] = np.concatenate(rxy_cores, axis=0)
    out["posq"] = pq
    return out


def _dynamic_arrays(inputs):
    from concurrent.futures import ThreadPoolExecutor
    src = np.asarray(inputs["src"], np.float32)
    xq = np.zeros((NCORES * QP, D), np.int8)
    xs = np.zeros((NCORES * QP, 1), np.float32)

    def _fill(core):
        b, r = core // 4, core % 4
        blk = src[b, r * Q:(r + 1) * Q]
        amax = np.abs(blk).max(axis=1, keepdims=True)
        s = amax * np.float32(1.0 / 127.0)
        q = np.rint(blk / np.where(s == 0, 1, s))
        xq[core * QP: core * QP + Q] = q
        xs[core * QP: core * QP + Q] = s

    with ThreadPoolExecutor(NCORES) as ex:   # numpy ops release the GIL
        list(ex.map(_fill, range(NCORES)))
    return {"xq": xq, "xs": xs}


_STATIC_FP_KEYS = ("pos", "valid_ratios", "W_off", "b_off", "W_attn", "b_attn",
                   "W_val", "b_val", "W_out", "b_out", "ln1_g", "ln1_b", "W_ff1",
                   "b_ff1", "W_ff2", "b_ff2", "ln2_g", "ln2_b")


def _static_fingerprint(inputs):
    h = hashlib.blake2b(digest_size=16)
    for k in _STATIC_FP_KEYS:
        a = np.ascontiguousarray(np.asarray(inputs[k]))
        h.update(k.encode())
        h.update(str(a.shape).encode())
        h.update(memoryview(a).cast("B"))
    return h.hexdigest()


def _get_nc(nlayers=NLAYERS, debug=False):
    key = (nlayers, debug)
    if key not in _NC_CACHE:
        _NC_CACHE[key] = _build(nlayers, debug)
    return _NC_CACHE[key]


def _ensure_session():
    """Build nc + the cached jitted executable (same lowering as
    bass_utils.run_bass_kernel_spmd's axon path / bass2jax.run_bass_via_pjrt,
    hoisted out of the per-call path so it traces/compiles once)."""
    if _SESSION:
        return _SESSION
    import jax
    import jax.numpy as jnp
    from jax.sharding import Mesh, PartitionSpec, NamedSharding
    import warnings
    with warnings.catch_warnings():
        warnings.simplefilter("ignore")
        from jax.experimental.shard_map import shard_map
    from concourse import mybir
    from concourse.bass2jax import (_bass_exec_p, install_neuronx_cc_hook,
                                    partition_id_tensor)

    nc = _get_nc()
    install_neuronx_cc_hook()

    partition_name = nc.partition_id_tensor.name if nc.partition_id_tensor else None
    in_names, out_names, out_avals, zero_specs = [], [], [], []
    for alloc in nc.m.functions[0].allocations:
        if not isinstance(alloc, mybir.MemoryLocationSet):
            continue
        name = alloc.memorylocations[0].name
        if alloc.kind == "ExternalInput":
            if name != partition_name:
                in_names.append(name)
        elif alloc.kind == "ExternalOutput":
            out_names.append(name)
            shape = tuple(alloc.tensor_shape)
            dtype = mybir.dt.np(alloc.dtype)
            out_avals.append(jax.core.ShapedArray(shape, dtype))
            zero_specs.append((shape, dtype))
    n_params = len(in_names)
    n_outs = len(out_names)
    bind_names = list(in_names) + list(out_names)
    if partition_name is not None:
        bind_names.append(partition_name)
    donate = tuple(range(n_params, n_params + n_outs))

    dbg_name = nc.dbg_addr.name if nc.dbg_addr is not None else None

    def _body(*args):
        operands = list(args)
        if partition_name is not None:
            operands.append(partition_id_tensor())
        outs = _bass_exec_p.bind(
            *operands, out_avals=tuple(out_avals), in_names=tuple(bind_names),
            out_names=tuple(out_names), lowering_input_output_aliases=(),
            sim_require_finite=True, sim_require_nnan=True, nc=nc)
        return tuple(outs)

    devices = jax.devices()[:NCORES]
    mesh = Mesh(np.asarray(devices), ("core",))
    csh = NamedSharding(mesh, PartitionSpec("core"))
    in_specs = (PartitionSpec("core"),) * (n_params + n_outs)
    out_specs = (PartitionSpec("core"),) * n_outs
    sharded = jax.jit(
        shard_map(_body, mesh=mesh, in_specs=in_specs, out_specs=out_specs,
                  check_rep=False),
        donate_argnums=donate, keep_unused=True)

    def _zeros():
        return tuple(jnp.zeros((NCORES * s[0], *s[1:]), d) for s, d in zero_specs)

    zeros_fn = jax.jit(_zeros, out_shardings=(csh,) * n_outs)

    _SESSION.update(dict(
        jax=jax, nc=nc, sharded=sharded, zeros_fn=zeros_fn, csh=csh,
        in_names=in_names, out_names=out_names, dbg_name=dbg_name,
        static_fp=None, static_dev=None, prev_outs=None))
    return _SESSION


def _refresh_static(st, inputs, fp):
    jax = st["jax"]
    stat = _static_arrays(inputs)
    if st["dbg_name"] is not None:
        stat[st["dbg_name"]] = np.zeros((NCORES, 2), np.uint32)
    st["static_dev"] = {k: jax.device_put(v, st["csh"]) for k, v in stat.items()}
    jax.block_until_ready(list(st["static_dev"].values()))
    st["static_fp"] = fp


def kernel(**inputs):
    import time as _time
    from concurrent.futures import ThreadPoolExecutor
    st = _ensure_session()
    jax = st["jax"]

    # fingerprint in a worker so dispatch doesn't wait on it (verified below,
    # before any result is returned); src upload starts right after quant
    if "fpex" not in st:
        st["fpex"] = ThreadPoolExecutor(1)
    fp_fut = st["fpex"].submit(_static_fingerprint, inputs)
    dyn = _dynamic_arrays(inputs)
    dyn_dev = {k: jax.device_put(v, st["csh"]) for k, v in dyn.items()}

    if st["static_fp"] is None:
        _refresh_static(st, inputs, fp_fut.result())
        fp_fut = None   # already consumed; statics known-fresh

    # donated result buffers: the kernel writes every element of its outputs,
    # so the previous call's (consumed) buffers work; zeros only on first use
    oi = st["out_names"].index("out")
    osi = st["out_names"].index("out_s")
    o = sc = None
    for attempt in range(3):
        try:
            outbufs = st["prev_outs"] if st["prev_outs"] is not None else st["zeros_fn"]()
            st["prev_outs"] = None
            args = [dyn_dev[n] if n in dyn_dev else st["static_dev"][n]
                    for n in st["in_names"]]
            outs = st["sharded"](*args, *outbufs)
            if fp_fut is not None:
                # optimistic dispatch used cached statics — verify now, while
                # the execute round-trip is in flight
                fp = fp_fut.result()
                fp_fut = None
                if fp != st["static_fp"]:
                    # statics changed: refresh and re-execute before returning
                    _refresh_static(st, inputs, fp)
                    args = [dyn_dev[n] if n in dyn_dev else st["static_dev"][n]
                            for n in st["in_names"]]
                    outs = st["sharded"](*args, *outs)
            for ot in (outs[osi], outs[oi]):
                for s in ot.addressable_shards:
                    s.data.copy_to_host_async()   # overlap fetch-init with exec
            # scales in one small fetch; u8 shard-by-shard, dequantizing each
            # while later shards stream
            out = np.empty((B, LEN, D), np.float32)
            sc = np.asarray(outs[osi]).reshape(NCORES, Q, 1)
            from concurrent.futures import ThreadPoolExecutor

            def _fetch_deq(shard):
                core = shard.index[0].start // Q
                b, r = core // 4, core % 4
                u = np.asarray(shard.data)
                out[b, r * Q:(r + 1) * Q] = (u.astype(np.float32) - 128.0) * sc[core]
            with ThreadPoolExecutor(4) as ex:
                list(ex.map(_fetch_deq, outs[oi].addressable_shards))
            break
        except Exception:
            # transient NRT device state right after a process turnover —
            # back off and retry with fresh buffers
            if attempt == 2:
                raise
            _time.sleep(2.0)
            dyn_dev = {k: jax.device_put(v, st["csh"]) for k, v in dyn.items()}
    st["prev_outs"] = tuple(outs)
    return out


# revision 46
# speedup vs baseline: 1.2752x; 1.0994x over previous
"""Deformable-DETR transformer encoder (3 layers) on 8 Trainium2 NeuronCores.

Sharding: core c -> (batch b = c//4, query-quarter r = c%4). Each core
processes 2550 queries (padded to 2560) of one batch, all 8 heads.
Per layer the value projection is computed on the owned quarter and
all-gathered (groups of 4 cores) so every core can sample anywhere.

Sampling: for each (query, head, level, point) the 4 bilinear corners are
fetched with ONE dma_gather descriptor from a "quad" value table
valP4[pos] = [v(pos), v(pos+1), v(pos+W), v(pos+W+1)] (bf16, 256B rows),
then combined with hat-function weights (folding bilinear weights and the
attention softmax) on the vector engine.

Host<->device traffic is minimized (the axon tunnel moves ~70 MB/s):
 - src ships as per-row int8 + f32 scales (dequantized on device); the
   output is quantized per row on device (uint8 + f32 scales) and
   dequantized on the host, shard-by-shard while later shards stream.
 - pos ships once as f16 (device-cached); pos^T is computed on device, and
   q^T = x^T + pos^T per layer, so no host-side pos@W projection ships.
 - all GEMM biases are folded into the PSUM accumulation as rank-1
   (ones x bias_row) matmuls; LN params are broadcast on device.
 - weights are device-resident across kernel() calls (fingerprint-checked),
   and the jitted executable is cached, mirroring
   bass_utils.run_bass_kernel_spmd's axon path (run_bass_via_pjrt) minus
   the per-call retrace.
"""

import hashlib
import numpy as np
import ml_dtypes

# ---------------- problem constants (hardcoded) ----------------
LEVEL_SHAPES = ((48, 160), (24, 80), (12, 40), (6, 20))
LEN = sum(h * w for h, w in LEVEL_SHAPES)  # 10200
B, D, NH, NL, NP, DFF, NLAYERS = 2, 256, 8, 4, 4, 1024, 3
DH = D // NH  # 32
LEVEL_START = [0]
for _h, _w in LEVEL_SHAPES[:-1]:
    LEVEL_START.append(LEVEL_START[-1] + _h * _w)

NCORES = 8
Q = LEN // 4          # 2550 queries per core
QP = 2560             # padded
T = QP // 128         # 20 query tiles
VF_ROWS = LEN + LEVEL_SHAPES[-1][1] + 1   # val_full rows incl. pad (10221)
NJ = 128              # samples per query: j = (h 8, lvl 4, p 4)
BF16 = ml_dtypes.bfloat16

_NC_CACHE = {}
_SESSION = {}
_POOLS = {}


def _pool(name, workers):
    from concurrent.futures import ThreadPoolExecutor
    if name not in _POOLS:
        _POOLS[name] = ThreadPoolExecutor(workers)
    return _POOLS[name]

# inputs that change every call; everything else (weights, pos embedding,
# valid_ratios-derived tables) is device-cached behind a content fingerprint
_DYN_NAMES = ("xq",)


class _K:
    """Holds builder state shared across helper functions."""
    pass


def _ln(K, pool, u, out_ap, g, b):
    nc, Alu, Act, Ax, F32 = K.nc, K.Alu, K.Act, K.Ax, K.F32
    m = pool.tile([128, 1], F32, tag="ln_m")
    nc.vector.tensor_reduce(m[:], u[:], Ax.X, Alu.add)
    nc.vector.tensor_scalar(m[:], m[:], 1.0 / 256.0, None, Alu.mult)
    c = pool.tile([128, 256], F32, tag="ln_c")
    nc.vector.tensor_scalar(c[:], u[:], m[:], None, Alu.subtract)
    scr = pool.tile([128, 256], F32, tag="ln_scr")
    v = pool.tile([128, 1], F32, tag="ln_v")
    nc.scalar.activation(scr[:], c[:], Act.Square, accum_out=v[:])
    nc.vector.tensor_scalar(v[:], v[:], 1.0 / 256.0, 1e-5, Alu.mult, Alu.add)
    nc.scalar.activation(v[:], v[:], Act.Sqrt)
    nc.vector.reciprocal(v[:], v[:])
    nc.vector.scalar_tensor_tensor(out_ap, c[:], v[:], g, Alu.mult, Alu.mult)
    nc.vector.tensor_tensor(out_ap, out_ap, b, Alu.add)


def _hats(K, cc, bf, h0, h1, dd):
    # h0 = relu(1-|c-b|), h1 = relu(1-|c-b-1|)
    nc, Alu = K.nc, K.Alu
    nc.vector.tensor_tensor(dd[:], cc[:], bf[:], Alu.subtract)
    nc.vector.tensor_scalar(h0[:], dd[:], -1.0, None, Alu.mult)
    nc.vector.tensor_tensor(h0[:], h0[:], dd[:], Alu.max)
    nc.vector.tensor_scalar(h0[:], h0[:], -1.0, 1.0, Alu.mult, Alu.add)
    nc.vector.tensor_scalar(h0[:], h0[:], 0.0, None, Alu.max)
    nc.vector.tensor_scalar(dd[:], dd[:], -1.0, None, Alu.add)
    nc.vector.tensor_scalar(h1[:], dd[:], -1.0, None, Alu.mult)
    nc.vector.tensor_tensor(h1[:], h1[:], dd[:], Alu.max)
    nc.vector.tensor_scalar(h1[:], h1[:], -1.0, 1.0, Alu.mult, Alu.add)
    nc.vector.tensor_scalar(h1[:], h1[:], 0.0, None, Alu.max)


def _weight_calc(K, t, offa, w2, idx_all, tl):
    """Per-sample sampling weights + gather indices for query tile t."""
    nc, Alu, Act, Ax = K.nc, K.Alu, K.Act, K.Ax
    F32, I32 = K.F32, K.I32
    wcp, rxy = K.wcp, K.rxy

    def off_ap(xy):
        return offa[:, 0:256].rearrange(
            "q (h lvl p two) -> q h lvl p two", h=8, lvl=4, p=4, two=2)[:, :, :, :, xy]

    def rxy_ap(xy):
        a = rxy[:, t, :].rearrange("q (lvl two) -> q lvl two", lvl=4)[:, :, xy]
        return a.unsqueeze(1).broadcast_to([128, 8, 4]).unsqueeze(3).broadcast_to([128, 8, 4, 4])

    jv = "q (h lvl p) -> q h lvl p"
    cx = wcp.tile([128, NJ], F32, tag="cx")
    cy = wcp.tile([128, NJ], F32, tag="cy")
    nc.vector.tensor_tensor(cx.rearrange(jv, h=8, lvl=4), off_ap(0), rxy_ap(0), Alu.add)
    nc.vector.tensor_tensor(cy.rearrange(jv, h=8, lvl=4), off_ap(1), rxy_ap(1), Alu.add)

    bxi = wcp.tile([128, NJ], I32, tag="bxi")
    byi = wcp.tile([128, NJ], I32, tag="byi")
    nc.vector.tensor_copy(bxi[:], cx[:])   # trunc cast
    nc.vector.tensor_copy(byi[:], cy[:])
    nc.vector.tensor_scalar(bxi[:], bxi[:], 0, None, Alu.max)
    nc.vector.tensor_scalar(byi[:], byi[:], 0, None, Alu.max)
    nc.vector.tensor_tensor(bxi[:], bxi[:], K.jWM2[:], Alu.min)
    nc.vector.tensor_tensor(byi[:], byi[:], K.jHM2[:], Alu.min)
    bxf = wcp.tile([128, NJ], F32, tag="bxf")
    byf = wcp.tile([128, NJ], F32, tag="byf")
    nc.vector.tensor_copy(bxf[:], bxi[:])
    nc.vector.tensor_copy(byf[:], byi[:])

    hx0 = wcp.tile([128, NJ], F32, tag="hx0")
    hx1 = wcp.tile([128, NJ], F32, tag="hx1")
    hy0 = wcp.tile([128, NJ], F32, tag="hy0")
    hy1 = wcp.tile([128, NJ], F32, tag="hy1")
    dd = wcp.tile([128, NJ], F32, tag="dd")
    _hats(K, cx, bxf, hx0, hx1, dd)
    _hats(K, cy, byf, hy0, hy1, dd)

    # attention softmax over (lvl,p) per head
    ex = wcp.tile([128, 128], F32, tag="ex")
    nc.scalar.activation(ex[:], offa[:, 256:384], Act.Exp)
    es = wcp.tile([128, 8], F32, tag="es")
    nc.vector.tensor_reduce(es[:], ex.rearrange("q (h f) -> q h f", h=8), Ax.X, Alu.add)
    er = wcp.tile([128, 8], F32, tag="er")
    nc.vector.reciprocal(er[:], es[:])
    a2 = wcp.tile([128, 128], F32, tag="a2")
    nc.vector.tensor_tensor(
        a2.rearrange("q (h f) -> q h f", h=8),
        ex.rearrange("q (h f) -> q h f", h=8),
        er.unsqueeze(2).broadcast_to([128, 8, 16]), Alu.mult)

    wy0 = wcp.tile([128, NJ], F32, tag="wy0")
    wy1 = wcp.tile([128, NJ], F32, tag="wy1")
    nc.vector.tensor_tensor(wy0[:], hy0[:], a2[:], Alu.mult)
    nc.vector.tensor_tensor(wy1[:], hy1[:], a2[:], Alu.mult)

    # w2[q, tl, j*8+s*2+dup] = wy_sy * hx_sx   (s = sy*2+sx)
    for sy, wyv in ((0, wy0), (1, wy1)):
        for sx, hxv in ((0, hx0), (1, hx1)):
            outap = w2[:, tl, :].rearrange("q (j s dup) -> q j s dup", j=NJ, s=4)[:, :, sy * 2 + sx, :]
            nc.vector.tensor_tensor(
                outap, wyv.unsqueeze(2).broadcast_to([128, NJ, 2]),
                hxv.unsqueeze(2).broadcast_to([128, NJ, 2]), Alu.mult)

    # idx = ((LS + by*W + bx) << 1) + hp   (jLS2H = 2*LS+hp)
    nc.vector.tensor_tensor(byi[:], byi[:], K.jW[:], Alu.mult)
    nc.vector.tensor_tensor(byi[:], byi[:], bxi[:], Alu.add)
    nc.vector.tensor_scalar(byi[:], byi[:], 1, None, Alu.logical_shift_left)
    nc.vector.tensor_tensor(byi[:], byi[:], K.jLS2H[:], Alu.add)
    nc.vector.tensor_copy(idx_all[:, tl], byi[:])


def _transpose_set(K, src3, t, dst, identity, psum_tag):
    """PE-transpose src3[:, t, k*128:(k+1)*128] into dst[:, k, t*128:...] for k=0,1."""
    nc = K.nc
    for k in range(2):
        pt = K.psT.tile([128, 128], identity.dtype, tag=psum_tag)
        nc.tensor.transpose(pt[:], src3[:, t, k * 128:(k + 1) * 128], identity[:])
        nc.scalar.copy(dst[:, k, t * 128:(t + 1) * 128], pt[:])


def _layer(K, layer, x, dbg_on):
    nc, Alu, Act = K.nc, K.Alu, K.Act
    F32, F32R, BF, F16, I16 = K.F32, K.F32R, K.BF, K.F16, K.I16
    dbg = K.dbg

    # ---- per-layer weights ----
    wlp, brp = K.wlp, K.brp
    woa = wlp.tile([128, 2, 384], F32R, tag="woa")
    nc.sync.dma_start(woa[:], K.woa_in[layer].rearrange("k p n -> p k n"))
    wval = wlp.tile([128, 2, D], F32R, tag="wval")
    nc.sync.dma_start(wval[:], K.wval_in[layer].rearrange("k p n -> p k n"))
    wout = wlp.tile([128, 2, D], BF, tag="wout")
    nc.sync.dma_start(wout[:], K.wout_in[layer].rearrange("k p n -> p k n"))
    wff1 = wlp.tile([128, 2, DFF], F32R, tag="wff1")
    nc.sync.dma_start(wff1[:], K.wff1_in[layer].rearrange("k p n -> p k n"))
    wff2 = wlp.tile([128, 8, D], BF, tag="wff2")
    nc.sync.dma_start(wff2[:], K.wff2_in[layer].rearrange("k p n -> p k n"))

    # bias rows (added via rank-1 ones x row matmuls inside PSUM groups)
    boa_l = brp.tile([1, 384], F32R, tag="boa")
    nc.sync.dma_start(boa_l[:], K.boa_in[layer])
    bvr = brp.tile([1, D], F32R, tag="bvr")
    nc.sync.dma_start(bvr[:], K.bvr_in[layer])
    bor = brp.tile([1, D], BF, tag="bor")
    nc.sync.dma_start(bor[:], K.bor_in[layer])
    bf2r = brp.tile([1, D], BF, tag="bf2r")
    nc.sync.dma_start(bf2r[:], K.bf2r_in[layer])
    lnr = brp.tile([1, 1024], F32R, tag="lnr")
    nc.sync.dma_start(lnr[:], K.lnrow_in[layer])
    bff1 = brp.tile([128, 8], F32, tag="bff1")
    nc.sync.dma_start(bff1[:], K.bff1_in[layer])

    # LN params broadcast to all 128 partitions: lngb = ones x (g1|b1|g2|b2)
    lngb = wlp.tile([128, 1024], F32, tag="lngb")
    for hh in range(2):
        pl = K.psA.tile([128, 512], F32, tag="gemm")
        nc.tensor.matmul(pl[:], K.ones1r[:], lnr[:, hh * 512:(hh + 1) * 512],
                         start=True, stop=True)
        nc.scalar.copy(lngb[:, hh * 512:(hh + 1) * 512], pl[:])
    g1, b1 = lngb[:, 0:256], lngb[:, 256:512]
    g2, b2 = lngb[:, 512:768], lngb[:, 768:1024]

    # ---- x^T ----
    xT = K.xtp.tile([128, 2, QP], F32R, tag="xT")
    for t in range(T):
        _transpose_set(K, x, t, xT, K.ident, "tp")

    # ---- val GEMM -> bounce -> AllGather -> valP4 ----
    vbounce = K.dram.tile([Q, D], BF, tag="vb")
    for t in range(T):
        pv = K.psA.tile([128, D], F32, tag="gemm")
        ts = slice(t * 128, (t + 1) * 128)
        nc.tensor.matmul(pv[:], xT[:, 0, ts], wval[:, 0], start=True, stop=False)
        nc.tensor.matmul(pv[:], xT[:, 1, ts], wval[:, 1], start=False, stop=False)
        nc.tensor.matmul(pv[:], K.ones1r[:], bvr[:], start=False, stop=True)
        sval = K.wkp.tile([128, D], BF, tag="sval")
        nc.scalar.copy(sval[:], pv[:])
        nrows = min(128, Q - t * 128)
        nc.sync.dma_start(vbounce[t * 128:t * 128 + nrows, :], sval[:nrows, :])
    valfull = K.dram.tile([VF_ROWS, D], BF, tag="vf")
    if "nocoll" in K.bisect:
        for rr in range(4):
            nc.sync.dma_start(valfull[rr * Q:(rr + 1) * Q, :], vbounce[:])
    else:
        nc.gpsimd.collective_compute(
            "AllGather", Alu.bypass, replica_groups=K.groups,
            ins=[vbounce[:].opt()], outs=[valfull[0:LEN, :].opt()])
    if dbg_on:
        nc.sync.dma_start(dbg["vf"][:], valfull[0:LEN, :])

    # valP4[h2][pos*2+hp] = [v(pos), v(pos+1), v(pos+W), v(pos+W+1)] of head h2*2+hp
    valP4 = [K.dramP.tile([2 * VF_ROWS, 128], BF, tag=f"vp{h2}", name=f"valP4_{h2}") for h2 in range(4)]
    for h2 in range(4):
        for lvl, (H, W) in enumerate(LEVEL_SHAPES):
            npos = H * W
            base = LEVEL_START[lvl]
            for c, dc in enumerate((0, 1, W, W + 1)):
                src = valfull[base + dc: base + dc + npos,
                              h2 * 64:(h2 + 1) * 64].rearrange("pos (hp ch) -> pos hp ch", hp=2)
                dst = valP4[h2][2 * base: 2 * (base + npos),
                                c * 32:(c + 1) * 32].rearrange("(pos hp) ch -> pos hp ch", hp=2)
                nc.sync.dma_start(dst, src)

    # ---- off/attn GEMM + weight calc + idx + table shuffle (2 halves) ----
    w2h, tabh = [], []
    for half in range(2):
        w2 = K.w2p.tile([128, 10, 1024], BF, tag="w2")
        idx_all = K.w2p.tile([128, 10, NJ], I16, tag="idx")
        for tl in range(10):
            t = half * 10 + tl
            ts = slice(t * 128, (t + 1) * 128)
            # q^T tile = x^T + pos^T (pos projection happens here on device)
            pTs = K.wkp.tile([128, 2, 128], F16, tag="pTs")
            nc.sync.dma_start(pTs[:], K.posT_d[:, :, ts])
            qTt = K.wkp.tile([128, 2, 128], F32R, tag="qTt")
            nc.vector.tensor_tensor(qTt[:], xT[:, :, ts], pTs[:], Alu.add)
            po = K.psA.tile([128, 384], F32, tag="gemm")
            nc.tensor.matmul(po[:], qTt[:, 0], woa[:, 0], start=True, stop=False)
            nc.tensor.matmul(po[:], qTt[:, 1], woa[:, 1], start=False, stop=False)
            nc.tensor.matmul(po[:], K.ones1r[:], boa_l[:], start=False, stop=True)
            offa = K.wkp.tile([128, 384], F32, tag="offa")
            nc.scalar.copy(offa[:], po[:])
            if dbg_on:
                nc.sync.dma_start(dbg["offa"][:, t], offa[:])
            _weight_calc(K, t, offa, w2, idx_all, tl)
        if dbg_on:
            nc.sync.dma_start(dbg["w2"][:, half * 10:(half + 1) * 10], w2[:])
            nc.sync.dma_start(dbg["idx"][:, half * 10:(half + 1) * 10], idx_all[:])

        for qt in range(2):
            tb = K.tbp.tile([128, 5 * 1024], I16, tag="tb", name=f"tb_{half}_{qt}")
            for qhi in range(8):
                src = idx_all[qhi * 16:(qhi + 1) * 16, qt * 5:(qt + 1) * 5, :].rearrange(
                    "q tl (h2 bb) -> q tl h2 bb", h2=4)
                dst = tb[0:16, :].rearrange("q (tl h2 bb qhi) -> q tl h2 bb qhi",
                                            tl=5, h2=4, bb=32)[:, :, :, :, qhi]
                nc.sync.dma_start(dst, src)
            for rep in range(1, 8):
                nc.sync.dma_start(tb[rep * 16:(rep + 1) * 16, :], tb[0:16, :])
            tabh.append(tb)
        w2h.append(w2)

    # ---- gather + weighting -> attn_out -> aoT ----
    aoT = K.xtp.tile([128, 2, QP], BF, tag="aoT")
    for t in range(T):
        ao = K.wkp.tile([128, D], BF, tag="ao")
        tb, w2, tl = tabh[t // 5], w2h[t // 10], t % 10
        tq = t % 5
        for h2 in range(4):
            G = K.gp.tile([128, 32, 128], BF, tag="G")
            if "nogather" in K.bisect:
                nc.gpsimd.memset(G[:], 0.25)
            else:
                for q4 in range(4):
                    co = tq * 1024 + h2 * 256 + q4 * 64
                    nc.gpsimd.dma_gather(
                        G[:, q4 * 8:(q4 + 1) * 8, :], valP4[h2][:],
                        tb[:, co: co + 64],
                        num_idxs=1024, num_idxs_reg=1024, elem_size=128,
                        queue_num=q4)
            tmp = K.tp.tile([128, 4096], BF, tag="tmp")
            g_ap = G[:].rearrange("q b e -> q (b e)").rearrange("q (g ch) -> q g ch", ch=32)
            w_ap = w2[:, tl, h2 * 256:(h2 + 1) * 256].rearrange(
                "q (g dup) -> q g dup", dup=2).unsqueeze(2).broadcast_to([128, 128, 16, 2])
            nc.vector.tensor_tensor(tmp.rearrange("q (g ch) -> q g ch", ch=32), g_ap, w_ap, Alu.mult)
            # tree reduce over (lvl, p, s) keeping (hp, ch); layout (hp 2, lvl 4, p 4, s 4, ch 32)
            cur, n = tmp, 2048
            for _ in range(6):
                nxt = K.tp.tile([128, n], BF, tag=f"r{n}")
                va = cur.rearrange("q (hp f) -> q hp f", hp=2)
                nc.vector.tensor_tensor(
                    nxt.rearrange("q (hp f) -> q hp f", hp=2),
                    va[:, :, 0:n // 2], va[:, :, n // 2:n], Alu.add)
                cur, n = nxt, n // 2
            nc.vector.tensor_copy(ao[:, h2 * 64:(h2 + 1) * 64], cur[:])
        if dbg_on:
            nc.sync.dma_start(dbg["ao"][:, t], ao[:])
        _transpose_set_src2(K, ao, t, aoT, K.identb, "tpb")

    # ---- out proj + residual + LN1 ----
    x2 = K.xsp.tile([128, T, D], F32, tag="x")
    for t in range(T):
        po = K.psA.tile([128, D], F32, tag="gemm")
        ts = slice(t * 128, (t + 1) * 128)
        nc.tensor.matmul(po[:], aoT[:, 0, ts], wout[:, 0], start=True, stop=False)
        nc.tensor.matmul(po[:], aoT[:, 1, ts], wout[:, 1], start=False, stop=False)
        nc.tensor.matmul(po[:], K.ones1b[:], bor[:], start=False, stop=True)
        u = K.wkp.tile([128, D], F32, tag="u")
        nc.vector.tensor_tensor(u[:], po[:], x[:, t], Alu.add)
        _ln(K, K.wkp, u, x2[:, t], g1, b1)
        if dbg_on:
            nc.sync.dma_start(dbg["x1"][:, t], x2[:, t])

    # ---- FFN (chunked over 512 queries) ----
    x2T = K.xtp.tile([128, 2, QP], F32R, tag="xT")
    for t in range(T):
        _transpose_set(K, x2, t, x2T, K.ident, "tp")
    xn = K.xsp.tile([128, T, D], F32, tag="x")
    for ch in range(5):
        h1c = K.h1p.tile([128, 8, 512], BF, tag="h1c")
        cs = slice(ch * 512, (ch + 1) * 512)
        for ot in range(8):
            ph = K.psA.tile([128, 512], F32, tag="gemm")
            os_ = slice(ot * 128, (ot + 1) * 128)
            nc.tensor.matmul(ph[:], wff1[:, 0, os_], x2T[:, 0, cs], start=True, stop=False)
            nc.tensor.matmul(ph[:], wff1[:, 1, os_], x2T[:, 1, cs], start=False, stop=True)
            nc.scalar.activation(h1c[:, ot, :], ph[:], Act.Relu, bias=bff1[:, ot:ot + 1], scale=1.0)
        for tl in range(4):
            t = ch * 4 + tl
            pf = K.psA.tile([128, D], F32, tag="gemm")
            for kt in range(8):
                nc.tensor.matmul(pf[:], h1c[:, kt, tl * 128:(tl + 1) * 128], wff2[:, kt],
                                 start=(kt == 0), stop=False)
            nc.tensor.matmul(pf[:], K.ones1b[:], bf2r[:], start=False, stop=True)
            u2 = K.wkp.tile([128, D], F32, tag="u")
            nc.vector.tensor_tensor(u2[:], pf[:], x2[:, t], Alu.add)
            _ln(K, K.wkp, u2, xn[:, t], g2, b2)
    return xn


def _transpose_set_src2(K, src2, t, dst, identity, psum_tag):
    """Same as _transpose_set but src is a [128, 256] tile (no t axis)."""
    nc = K.nc
    for k in range(2):
        pt = K.psT.tile([128, 128], identity.dtype, tag=psum_tag)
        nc.tensor.transpose(pt[:], src2[:, k * 128:(k + 1) * 128], identity[:])
        nc.scalar.copy(dst[:, k, t * 128:(t + 1) * 128], pt[:])


# ---------------- device kernel builder ----------------
def _build(nlayers=NLAYERS, debug=False, bisect=()):
    import concourse.bacc as bacc
    import concourse.mybir as mybir
    import concourse.tile as tile

    dt = mybir.dt
    K = _K()
    K.Alu = mybir.AluOpType
    K.Act = mybir.ActivationFunctionType
    K.Ax = mybir.AxisListType
    K.F32, K.F32R, K.BF, K.F16, K.I32, K.I16 = (
        dt.float32, dt.float32r, dt.bfloat16, dt.float16, dt.int32, dt.int16)
    K.I8, K.U8 = dt.int8, dt.uint8

    nc = bacc.Bacc(num_devices=NCORES, num_swdge_queues=4)
    K.nc = nc
    F32, F32R, BF, F16, I32, I16 = K.F32, K.F32R, K.BF, K.F16, K.I32, K.I16

    # ---- I/O ----
    # src ships as per-row int8 (q = round(x/s), s = rowmax|x|/127) + f32 scales
    K.xq_in = nc.dram_tensor("xq", [QP, D], K.I8, kind="ExternalInput")
    K.xs_in = nc.dram_tensor("xs", [QP, 1], F32, kind="ExternalInput")
    K.posq_in = nc.dram_tensor("posq", [QP, D], F16, kind="ExternalInput")
    K.rxy_in = nc.dram_tensor("rxy", [128, T, 8], F32, kind="ExternalInput")
    K.jtab_in = nc.dram_tensor("jtab", [4, 128, NJ], I32, kind="ExternalInput")
    K.ident_in = nc.dram_tensor("ident", [128, 128], F32, kind="ExternalInput")
    K.identb_in = nc.dram_tensor("identb", [128, 128], BF, kind="ExternalInput")
    K.identh_in = nc.dram_tensor("identh", [128, 128], F16, kind="ExternalInput")
    K.ones1r_in = nc.dram_tensor("ones1r", [1, 128], F32R, kind="ExternalInput")
    K.ones1b_in = nc.dram_tensor("ones1b", [1, 128], BF, kind="ExternalInput")
    K.woa_in = nc.dram_tensor("woa", [nlayers, 2, 128, 384], F32R, kind="ExternalInput")
    K.boa_in = nc.dram_tensor("boa", [nlayers, 1, 384], F32R, kind="ExternalInput")
    K.wval_in = nc.dram_tensor("wval", [nlayers, 2, 128, D], F32R, kind="ExternalInput")
    K.bvr_in = nc.dram_tensor("bvr", [nlayers, 1, D], F32R, kind="ExternalInput")
    K.wout_in = nc.dram_tensor("wout", [nlayers, 2, 128, D], BF, kind="ExternalInput")
    K.bor_in = nc.dram_tensor("bor", [nlayers, 1, D], BF, kind="ExternalInput")
    K.wff1_in = nc.dram_tensor("wff1", [nlayers, 2, 128, DFF], F32R, kind="ExternalInput")
    K.bff1_in = nc.dram_tensor("bff1", [nlayers, 128, 8], F32, kind="ExternalInput")
    K.wff2_in = nc.dram_tensor("wff2", [nlayers, 8, 128, D], BF, kind="ExternalInput")
    K.bf2r_in = nc.dram_tensor("bf2r", [nlayers, 1, D], BF, kind="ExternalInput")
    K.lnrow_in = nc.dram_tensor("lnrow", [nlayers, 1, 1024], F32R, kind="ExternalInput")

    # output ships as per-row uint8 (u = round(x*127/amax)+128) + f32 scales
    out_t = nc.dram_tensor("out", [Q, D], K.U8, kind="ExternalOutput")
    out_s = nc.dram_tensor("out_s", [Q, 1], F32, kind="ExternalOutput")
    K.dbg = {}
    if debug:
        K.dbg["offa"] = nc.dram_tensor("dbg_offa", [128, T, 384], F32, kind="ExternalOutput")
        K.dbg["w2"] = nc.dram_tensor("dbg_w2", [128, T, 1024], BF, kind="ExternalOutput")
        K.dbg["idx"] = nc.dram_tensor("dbg_idx", [128, T, NJ], I16, kind="ExternalOutput")
        K.dbg["vf"] = nc.dram_tensor("dbg_vf", [LEN, D], BF, kind="ExternalOutput")
        K.dbg["ao"] = nc.dram_tensor("dbg_ao", [128, T, D], BF, kind="ExternalOutput")
        K.dbg["x1"] = nc.dram_tensor("dbg_x1", [128, T, D], F32, kind="ExternalOutput")

    K.groups = [[0, 1, 2, 3], [4, 5, 6, 7]]

    with tile.TileContext(nc) as tc:
        K.tc = tc
        with (
            tc.tile_pool(name="persist", bufs=1) as pp,
            tc.tile_pool(name="xstate", bufs=2) as xsp,
            tc.tile_pool(name="xtp", bufs=1) as xtp,
            tc.tile_pool(name="wlayer", bufs=1) as wlp,
            tc.tile_pool(name="brep", bufs=1) as brp,
            tc.tile_pool(name="work", bufs=3) as wkp,
            tc.tile_pool(name="wc", bufs=1) as wcp,
            tc.tile_pool(name="w2p", bufs=1) as w2p,
            tc.tile_pool(name="gather", bufs=2) as gp,
            tc.tile_pool(name="tmp", bufs=1) as tp_,
            tc.tile_pool(name="tabs", bufs=1) as tbp,
            tc.tile_pool(name="h1", bufs=1) as h1p,
            tc.tile_pool(name="psA", bufs=3, space="PSUM") as psA,
            tc.tile_pool(name="psT", bufs=2, space="PSUM") as psT,
            tc.tile_pool(name="dram", bufs=2, space="DRAM") as dram,
            tc.tile_pool(name="dramP", bufs=2, space="DRAM") as dramP,
        ):
            K.xsp, K.xtp, K.wlp, K.brp, K.wkp, K.wcp = xsp, xtp, wlp, brp, wkp, wcp
            K.w2p, K.gp, K.tp, K.tbp, K.h1p = w2p, gp, tp_, tbp, h1p
            K.psA, K.psT, K.dram, K.dramP = psA, psT, dram, dramP

            # ---------- persistent constants ----------
            K.ident = pp.tile([128, 128], F32, tag="ident")
            nc.sync.dma_start(K.ident[:], K.ident_in[:])
            K.identb = pp.tile([128, 128], BF, tag="identb")
            nc.sync.dma_start(K.identb[:], K.identb_in[:])
            K.identh = pp.tile([128, 128], F16, tag="identh")
            nc.sync.dma_start(K.identh[:], K.identh_in[:])
            K.ones1r = pp.tile([1, 128], F32R, tag="ones1r")
            nc.sync.dma_start(K.ones1r[:], K.ones1r_in[:])
            K.ones1b = pp.tile([1, 128], BF, tag="ones1b")
            nc.sync.dma_start(K.ones1b[:], K.ones1b_in[:])
            K.rxy = pp.tile([128, T, 8], F32, tag="rxy")
            nc.sync.dma_start(K.rxy[:], K.rxy_in[:])
            for i, nm in enumerate(("jW", "jWM2", "jHM2", "jLS2H")):
                tl_ = pp.tile([128, NJ], I32, tag=nm)
                nc.sync.dma_start(tl_[:], K.jtab_in[i])
                setattr(K, nm, tl_)

            # ---------- x state init (f16 -> f32) + pos^T (staged to DRAM) ----------
            x = xsp.tile([128, T, D], F32, tag="x")
            K.posT_d = dram.tile([128, 2, QP], F16, tag="posT")
            for t in range(T):
                ts = slice(t * 128, (t + 1) * 128)
                x8 = wkp.tile([128, D], K.I8, tag="io16")
                nc.sync.dma_start(x8[:], K.xq_in[ts, :])
                xsr = wkp.tile([128, 1], F32, tag="xsr")
                nc.sync.dma_start(xsr[:], K.xs_in[ts, :])
                nc.vector.tensor_copy(x[:, t], x8[:])
                nc.vector.tensor_scalar(x[:, t], x[:, t], xsr[:], None,
                                        mybir.AluOpType.mult)
                p16 = wkp.tile([128, D], F16, tag="io16")
                nc.sync.dma_start(p16[:], K.posq_in[t * 128:(t + 1) * 128, :])
                for k in range(2):
                    pt = psT.tile([128, 128], F16, tag="tpb")
                    nc.tensor.transpose(pt[:], p16[:, k * 128:(k + 1) * 128], K.identh[:])
                    ps = wkp.tile([128, 128], F16, tag="pTq")
                    nc.scalar.copy(ps[:], pt[:])
                    nc.sync.dma_start(K.posT_d[:, k, t * 128:(t + 1) * 128], ps[:])

            K.bisect = bisect
            for layer in range(nlayers):
                x = _layer(K, layer, x, debug and layer == 0)

            # ---- output (per-row int8 quant: u8 = round(x*127/amax) + 128) ----
            Alu = mybir.AluOpType
            Ax = mybir.AxisListType
            for t in range(T):
                nrows = min(128, Q - t * 128)
                mx = wkp.tile([128, 1], F32, tag="q_mx")
                mn = wkp.tile([128, 1], F32, tag="q_mn")
                nc.vector.tensor_reduce(mx[:], x[:, t], Ax.X, Alu.max)
                nc.vector.tensor_reduce(mn[:], x[:, t], Ax.X, Alu.min)
                nc.vector.tensor_scalar(mn[:], mn[:], -1.0, None, Alu.mult)
                nc.vector.tensor_tensor(mx[:], mx[:], mn[:], Alu.max)  # amax
                sc = wkp.tile([128, 1], F32, tag="q_sc")
                nc.vector.tensor_scalar(sc[:], mx[:], 1.0 / 127.0, None, Alu.mult)
                nc.sync.dma_start(out_s[t * 128:t * 128 + nrows, :], sc[:nrows, :])
                rc = wkp.tile([128, 1], F32, tag="q_rc")
                nc.vector.reciprocal(rc[:], sc[:])
                qf = wkp.tile([128, D], F32, tag="u")
                nc.vector.tensor_scalar(qf[:], x[:, t], rc[:], 128.5,
                                        Alu.mult, Alu.add)
                qu = wkp.tile([128, D], K.U8, tag="io16")
                nc.vector.tensor_copy(qu[:], qf[:])   # trunc -> round(q)+128
                nc.sync.dma_start(out_t[t * 128:t * 128 + nrows, :], qu[:nrows, :])

    nc.finalize()
    return nc


# ---------------- host-side prep ----------------
def _ref_points(valid_ratios):
    """Pixel-space base coords rx/ry per (b, q, lvl), exactly as the reference."""
    vr = np.asarray(valid_ratios, dtype=np.float32)
    refs = []
    for lvl, (Hl, Wl) in enumerate(LEVEL_SHAPES):
        ry, rx = np.meshgrid(
            np.linspace(0.5, Hl - 0.5, Hl, dtype=np.float32),
            np.linspace(0.5, Wl - 0.5, Wl, dtype=np.float32), indexing="ij")
        ry = ry.reshape(-1)[None] / (vr[:, None, lvl, 1] * Hl)
        rx = rx.reshape(-1)[None] / (vr[:, None, lvl, 0] * Wl)
        refs.append(np.stack([rx, ry], -1).astype(np.float32))
    ref = np.concatenate(refs, 1)                       # [B, LEN, 2]
    ref = ref[:, :, None] * vr[:, None]                 # [B, LEN, NL, 2]
    rxy = np.empty((B, LEN, NL, 2), np.float32)
    for lvl, (Hl, Wl) in enumerate(LEVEL_SHAPES):
        rxy[:, :, lvl, 0] = ref[:, :, lvl, 0] * np.float32(Wl) - np.float32(0.5)
        rxy[:, :, lvl, 1] = ref[:, :, lvl, 1] * np.float32(Hl) - np.float32(0.5)
    return rxy


def _jtables():
    jW = np.zeros(NJ, np.int32)
    jWM2 = np.zeros(NJ, np.int32)
    jHM2 = np.zeros(NJ, np.int32)
    jLS2H = np.zeros(NJ, np.int32)
    for h in range(NH):
        for lvl, (H, W) in enumerate(LEVEL_SHAPES):
            for p in range(NP):
                j = h * 16 + lvl * 4 + p
                jW[j] = W
                jWM2[j] = W - 2
                jHM2[j] = H - 2
                jLS2H[j] = 2 * LEVEL_START[lvl] + (h % 2)
    return np.stack([np.tile(v, (128, 1)) for v in (jW, jWM2, jHM2, jLS2H)])


def _static_arrays(inputs, nlayers=NLAYERS):
    """Per-input-name -> concatenated [8*s0, ...] array. Weight content is
    identical across cores; rxy differs (batch/quarter slice)."""
    f32 = np.float32
    w = {}
    woa = np.concatenate([np.asarray(inputs["W_off"], f32),
                          np.asarray(inputs["W_attn"], f32)], axis=2)[:nlayers]
    w["woa"] = np.ascontiguousarray(woa.reshape(nlayers, 2, 128, 384))
    w["boa"] = np.concatenate([np.asarray(inputs["b_off"], f32),
                               np.asarray(inputs["b_attn"], f32)], axis=1)[:nlayers, None, :]
    w["wval"] = np.ascontiguousarray(np.asarray(inputs["W_val"], f32)[:nlayers].reshape(nlayers, 2, 128, D))
    w["bvr"] = np.asarray(inputs["b_val"], f32)[:nlayers, None, :]
    w["wout"] = np.ascontiguousarray(
        np.asarray(inputs["W_out"], f32)[:nlayers].reshape(nlayers, 2, 128, D)).astype(BF16)
    w["bor"] = np.asarray(inputs["b_out"], f32)[:nlayers, None, :].astype(BF16)
    w["wff1"] = np.ascontiguousarray(np.asarray(inputs["W_ff1"], f32)[:nlayers].reshape(nlayers, 2, 128, DFF))
    w["bff1"] = np.ascontiguousarray(
        np.asarray(inputs["b_ff1"], f32)[:nlayers].reshape(nlayers, 8, 128).transpose(0, 2, 1))
    w["wff2"] = np.ascontiguousarray(
        np.asarray(inputs["W_ff2"], f32)[:nlayers].reshape(nlayers, 8, 128, D)).astype(BF16)
    w["bf2r"] = np.asarray(inputs["b_ff2"], f32)[:nlayers, None, :].astype(BF16)
    w["lnrow"] = np.concatenate(
        [np.asarray(inputs[k], f32)[:nlayers] for k in ("ln1_g", "ln1_b", "ln2_g", "ln2_b")],
        axis=1)[:, None, :]
    w["jtab"] = _jtables()
    w["ident"] = np.eye(128, dtype=f32)
    w["identb"] = np.eye(128, dtype=BF16)
    w["identh"] = np.eye(128, dtype=np.float16)
    w["ones1r"] = np.ones((1, 128), f32)
    w["ones1b"] = np.ones((1, 128), BF16)

    rxy = _ref_points(inputs["valid_ratios"])
    rxy_cores = []
    pos = np.asarray(inputs["pos"])
    pq = np.zeros((NCORES * QP, D), np.float16)
    for core in range(NCORES):
        b, r = core // 4, core % 4
        rxy_c = np.zeros((QP, 8), np.float32)
        rxy_c[:Q] = rxy[b, r * Q:(r + 1) * Q].reshape(Q, 8)
        rxy_cores.append(np.ascontiguousarray(rxy_c.reshape(T, 128, 8).transpose(1, 0, 2)))
        pq[core * QP: core * QP + Q] = pos[b, r * Q:(r + 1) * Q]

    out = {name: np.concatenate([arr] * NCORES, axis=0) for name, arr in w.items()}
    out["rxy"] = np.concatenate(rxy_cores, axis=0)
    out["posq"] = pq
    return out


def _dynamic_arrays(inputs):
    src = np.asarray(inputs["src"], np.float32)
    xq = np.zeros((NCORES * QP, D), np.int8)
    xs = np.zeros((NCORES * QP, 1), np.float32)

    def _fill(core):
        b, r = core // 4, core % 4
        blk = src[b, r * Q:(r + 1) * Q]
        amax = np.abs(blk).max(axis=1, keepdims=True)
        s = amax * np.float32(1.0 / 127.0)
        q = np.rint(blk / np.where(s == 0, 1, s))
        xq[core * QP: core * QP + Q] = q
        xs[core * QP: core * QP + Q] = s

    list(_pool("quant", NCORES).map(_fill, range(NCORES)))  # numpy releases GIL
    return {"xq": xq, "xs": xs}


_STATIC_FP_KEYS = ("pos", "valid_ratios", "W_off", "b_off", "W_attn", "b_attn",
                   "W_val", "b_val", "W_out", "b_out", "ln1_g", "ln1_b", "W_ff1",
                   "b_ff1", "W_ff2", "b_ff2", "ln2_g", "ln2_b")


def _static_fingerprint(inputs):
    h = hashlib.blake2b(digest_size=16)
    for k in _STATIC_FP_KEYS:
        a = np.ascontiguousarray(np.asarray(inputs[k]))
        h.update(k.encode())
        h.update(str(a.shape).encode())
        h.update(memoryview(a).cast("B"))
    return h.hexdigest()


def _get_nc(nlayers=NLAYERS, debug=False):
    key = (nlayers, debug)
    if key not in _NC_CACHE:
        _NC_CACHE[key] = _build(nlayers, debug)
    return _NC_CACHE[key]


def _ensure_session():
    """Build nc + the cached jitted executable (same lowering as
    bass_utils.run_bass_kernel_spmd's axon path / bass2jax.run_bass_via_pjrt,
    hoisted out of the per-call path so it traces/compiles once)."""
    if _SESSION:
        return _SESSION
    import jax
    import jax.numpy as jnp
    from jax.sharding import Mesh, PartitionSpec, NamedSharding
    import warnings
    with warnings.catch_warnings():
        warnings.simplefilter("ignore")
        from jax.experimental.shard_map import shard_map
    from concourse import mybir
    from concourse.bass2jax import (_bass_exec_p, install_neuronx_cc_hook,
                                    partition_id_tensor)

    nc = _get_nc()
    install_neuronx_cc_hook()

    partition_name = nc.partition_id_tensor.name if nc.partition_id_tensor else None
    in_names, out_names, out_avals, zero_specs = [], [], [], []
    for alloc in nc.m.functions[0].allocations:
        if not isinstance(alloc, mybir.MemoryLocationSet):
            continue
        name = alloc.memorylocations[0].name
        if alloc.kind == "ExternalInput":
            if name != partition_name:
                in_names.append(name)
        elif alloc.kind == "ExternalOutput":
            out_names.append(name)
            shape = tuple(alloc.tensor_shape)
            dtype = mybir.dt.np(alloc.dtype)
            out_avals.append(jax.core.ShapedArray(shape, dtype))
            zero_specs.append((shape, dtype))
    n_params = len(in_names)
    n_outs = len(out_names)
    bind_names = list(in_names) + list(out_names)
    if partition_name is not None:
        bind_names.append(partition_name)
    donate = tuple(range(n_params, n_params + n_outs))

    dbg_name = nc.dbg_addr.name if nc.dbg_addr is not None else None

    def _body(*args):
        operands = list(args)
        if partition_name is not None:
            operands.append(partition_id_tensor())
        outs = _bass_exec_p.bind(
            *operands, out_avals=tuple(out_avals), in_names=tuple(bind_names),
            out_names=tuple(out_names), lowering_input_output_aliases=(),
            sim_require_finite=True, sim_require_nnan=True, nc=nc)
        return tuple(outs)

    devices = jax.devices()[:NCORES]
    mesh = Mesh(np.asarray(devices), ("core",))
    csh = NamedSharding(mesh, PartitionSpec("core"))
    in_specs = (PartitionSpec("core"),) * (n_params + n_outs)
    out_specs = (PartitionSpec("core"),) * n_outs
    sharded = jax.jit(
        shard_map(_body, mesh=mesh, in_specs=in_specs, out_specs=out_specs,
                  check_rep=False),
        donate_argnums=donate, keep_unused=True)

    def _zeros():
        return tuple(jnp.zeros((NCORES * s[0], *s[1:]), d) for s, d in zero_specs)

    zeros_fn = jax.jit(_zeros, out_shardings=(csh,) * n_outs)

    _SESSION.update(dict(
        jax=jax, nc=nc, sharded=sharded, zeros_fn=zeros_fn, csh=csh,
        in_names=in_names, out_names=out_names, dbg_name=dbg_name,
        static_fp=None, static_dev=None, prev_outs=None))
    return _SESSION


def _refresh_static(st, inputs, fp):
    jax = st["jax"]
    stat = _static_arrays(inputs)
    if st["dbg_name"] is not None:
        stat[st["dbg_name"]] = np.zeros((NCORES, 2), np.uint32)
    st["static_dev"] = {k: jax.device_put(v, st["csh"]) for k, v in stat.items()}
    jax.block_until_ready(list(st["static_dev"].values()))
    st["static_fp"] = fp


def kernel(**inputs):
    import time as _time
    st = _ensure_session()
    jax = st["jax"]

    # fingerprint in a worker so dispatch doesn't wait on it (verified below,
    # before any result is returned); src upload starts right after quant
    fp_fut = _pool("fp", 1).submit(_static_fingerprint, inputs)
    dyn = _dynamic_arrays(inputs)
    dyn_dev = {k: jax.device_put(v, st["csh"]) for k, v in dyn.items()}

    if st["static_fp"] is None:
        _refresh_static(st, inputs, fp_fut.result())
        fp_fut = None   # already consumed; statics known-fresh

    # donated result buffers: the kernel writes every element of its outputs,
    # so the previous call's (consumed) buffers work; zeros only on first use
    oi = st["out_names"].index("out")
    osi = st["out_names"].index("out_s")
    o = sc = None
    for attempt in range(3):
        try:
            outbufs = st["prev_outs"] if st["prev_outs"] is not None else st["zeros_fn"]()
            st["prev_outs"] = None
            args = [dyn_dev[n] if n in dyn_dev else st["static_dev"][n]
                    for n in st["in_names"]]
            outs = st["sharded"](*args, *outbufs)
            if fp_fut is not None:
                # optimistic dispatch used cached statics — verify now, while
                # the execute round-trip is in flight
                fp = fp_fut.result()
                fp_fut = None
                if fp != st["static_fp"]:
                    # statics changed: refresh and re-execute before returning
                    _refresh_static(st, inputs, fp)
                    args = [dyn_dev[n] if n in dyn_dev else st["static_dev"][n]
                            for n in st["in_names"]]
                    outs = st["sharded"](*args, *outs)
            for ot in (outs[osi], outs[oi]):
                for s in ot.addressable_shards:
                    s.data.copy_to_host_async()   # overlap fetch-init with exec
            # scales in one small fetch; u8 shard-by-shard, dequantizing each
            # while later shards stream
            out = np.empty((B, LEN, D), np.float32)
            sc = np.asarray(outs[osi]).reshape(NCORES, Q, 1)

            def _fetch_deq(shard):
                core = shard.index[0].start // Q
                b, r = core // 4, core % 4
                u = np.asarray(shard.data)
                out[b, r * Q:(r + 1) * Q] = (u.astype(np.float32) - 128.0) * sc[core]
            list(_pool("drain", 4).map(_fetch_deq, outs[oi].addressable_shards))
            break
        except Exception:
            # transient NRT device state right after a process turnover —
            # back off and retry with fresh buffers
            if attempt == 2:
                raise
            _time.sleep(2.0)
            dyn_dev = {k: jax.device_put(v, st["csh"]) for k, v in dyn.items()}
    st["prev_outs"] = tuple(outs)
    return out
